# revision 46
# baseline (speedup 1.0000x reference)
"""Multi-head causal attention (B=2, S=2048, D=1024, H=16) on 8 Trainium2 cores.

Sharding: tensor-parallel over heads. Core c computes QKV projection, causal
attention and softmax for heads {2c, 2c+1} over both batches, then an AllToAll
redistributes the attention output so core c owns rows [512c, 512c+512) of the
flattened (B*S, D) activation; each core applies the full output projection to
its row slice. Host code only slices/transposes inputs and concatenates the
per-core output slices.

All matmuls run in bf16 with fp32 PSUM accumulation. The pipeline works in
transposed layout ([dim, seq]) so that softmax reduces over the PSUM partition
axis via a ones-column folded into the PV matmul, and the attention output
lands directly in the layout the output projection consumes.

Key scheduling decisions (measured on HW via same-process A/B):
- Causal masking is additive, done on the PE (identity x mask-pattern matmul
  accumulated onto diagonal score blocks before exp), so the DVE is not on
  the scores->exp->PV critical path and exp underflows masked entries to 0.
- PV matmuls column-slice past fully-masked query prefixes (no memsets).
- The per-strip softmax normalization evacuates the PV psum accumulators to
  SBUF with two DVE copies first, freeing the banks for the next strip; the
  recip/gpsimd-broadcast/mul chain then runs off the critical path.
- Batch 1's QKV chains are deferred into closures fed into batch 0's
  (ACT-bound) attention stream at strip tails, filling PE idle windows.
- Output projection runs ob-outer/j-inner in 4 two-chain waves through the
  rotating score psum slots.
"""
import numpy as np
from contextlib import ExitStack

import jax
import ml_dtypes

import concourse.bass as bass
import concourse.tile as tile
from concourse import bacc, mybir
from concourse.bass2jax import (
    _bass_exec_p,
    install_neuronx_cc_hook,
    partition_id_tensor,
)
from jax.sharding import Mesh, PartitionSpec
from jax.experimental.shard_map import shard_map

B, S, D, H = 2, 2048, 1024, 16
DH = D // H            # 64
NCORES = 8
HPC = H // NCORES      # heads per core = 2
HD = HPC * DH          # head dims per core = 128
R = B * S              # flattened rows = 4096
RPC = R // NCORES      # rows per core after AllToAll = 512
QB = 512               # query block (also the AllToAll shard size)
KB = 128               # key block
NQB = S // QB          # 4 query blocks per batch
NKB = S // KB          # 16 key blocks per batch
CCH = D // 128         # contraction chunks for D-wide matmuls = 8

BF16 = mybir.dt.bfloat16
F32 = mybir.dt.float32
AF = mybir.ActivationFunctionType
ALU = mybir.AluOpType


def _build(causal: bool, repeat: int = 1, loop_n: int = 0,
           a2a_local: bool = False, parts: str = "full"):
    """Emit the SPMD Bass program (identical on all 8 cores).

    loop_n > 0 builds a timing variant: the whole per-iteration body runs
    inside a hardware For_i loop and the AllToAll is replaced by a local DMA
    copy (collectives cannot sit inside control flow), with the real output
    replaced by a tiny dummy (so the timing loop's donated output buffers are
    negligible to transfer). Used only to measure per-iteration device time.
    """
    timing = loop_n > 0
    nc = bacc.Bacc("TRN2", target_bir_lowering=False, debug=False,
                   num_devices=NCORES)

    xt = nc.dram_tensor("xt", [D, R], BF16, kind="ExternalInput").ap()
    wit = nc.dram_tensor("wit", [D, 3 * HD], BF16, kind="ExternalInput").ap()
    bi_s = nc.dram_tensor("bi_s", [3 * HD], F32, kind="ExternalInput").ap()
    wot = nc.dram_tensor("wot", [D, D], BF16, kind="ExternalInput").ap()
    bo_f = nc.dram_tensor("bo_f", [D], F32, kind="ExternalInput").ap()
    masks = nc.dram_tensor("masks", [KB, KB], BF16, kind="ExternalInput").ap()
    if timing:
        out_t = nc.dram_tensor("out_scratch", [D, RPC], F32).ap()
        dummy = nc.dram_tensor("tiny_out", [1, 16], F32, kind="ExternalOutput").ap()
    else:
        out_t = nc.dram_tensor("out_t", [D, RPC], F32, kind="ExternalOutput").ap()

    with tile.TileContext(nc) as tc, ExitStack() as octx:
        persist = octx.enter_context(tc.tile_pool(name="persist", bufs=1))
        dram = octx.enter_context(tc.tile_pool(name="dram", bufs=1, space="DRAM"))

        # ---- persistent SBUF state (x chunks queued right after wit: the
        # QKV matmuls need them first; wot/bo only matter at the end) ----
        wit_sb = persist.tile([128, CCH, 3 * HD], BF16)
        nc.sync.dma_start(wit_sb[:], wit.rearrange("(cc p) n -> p cc n", p=128))
        bias_sb = persist.tile([128, 3], F32)
        nc.sync.dma_start(bias_sb[:], bi_s.rearrange("(t p) -> p t", p=128))
        xt_pool = octx.enter_context(tc.tile_pool(name="xt_pool", bufs=1))
        xt_sb = xt_pool.tile([128, CCH, R], BF16)
        xt_r = xt.rearrange("(cc p) r -> p cc r", p=128)
        for cc in range(CCH):
            nc.sync.dma_start(xt_sb[:, cc, :], xt_r[:, cc, :])
        wot_sb = persist.tile([128, CCH, D], BF16)
        nc.sync.dma_start(wot_sb[:], wot.rearrange("(cc p) o -> p cc o", p=128))
        bo_sb = persist.tile([128, CCH], F32)
        nc.sync.dma_start(bo_sb[:], bo_f.rearrange("(oc p) -> p oc", p=128))
        # one [128,128] triangle: mask[i, j] = (j >= i), same for every
        # diagonal sub-block once the exp is column-sliced
        mask_sb = persist.tile([128, KB], BF16)
        if causal:
            nc.sync.dma_start(mask_sb[:], masks[:])

        identity = persist.tile([128, 128], BF16)
        from concourse.masks import make_identity
        make_identity(nc, identity[:])
        ones_row = persist.tile([1, 128], BF16)
        nc.vector.memset(ones_row[:], 1.0)

        # qT/kT: [head-dims (2 heads x 64), S] per batch; v: [k rows, 65] blocks
        qt_sb = [persist.tile([128, S], BF16, name=f"qt{b}") for b in range(B)]
        kt_sb = [persist.tile([128, S], BF16, name=f"kt{b}") for b in range(B)]
        # v_sb[h][:, g, 0:64] = v rows for global k-block g; col 64 = 1.0
        v_sb = [persist.tile([128, B * NKB, DH + 1], BF16, name=f"v{h}")
                for h in range(HPC)]
        for h in range(HPC):
            nc.vector.memset(v_sb[h][:, :, DH:DH + 1], 1.0)

        a2a_in = dram.tile([NCORES, HD, RPC], BF16)
        a2a_out = dram.tile([NCORES, HD, RPC], BF16)
        ao_sb = persist.tile([128, NCORES, RPC], BF16, name="ao_sb")

        # one PSUM pool for all phases; tags share slots so cross-phase reuse
        # only waits on the previous user of one slot, not a whole phase.
        # banks: big0(2) + big1(2) + tr(2) + o0(1) + o1(1) = 8
        psum = octx.enter_context(tc.tile_pool(name="psum", bufs=1,
                                               space="PSUM"))
        work = octx.enter_context(tc.tile_pool(name="work", bufs=3))
        epool = octx.enter_context(tc.tile_pool(name="epool", bufs=4))

        def big_ps(i, name):
            # [128, 1024] = 2 PSUM banks; halves are used as two 512-wide
            # chains / score blocks so ACT+DVE consumers see one wide AP.
            # One tag, 3 rotating slots (6 banks): the head-interleaved score
            # stream gets effective double-buffering without per-head tags.
            return psum.tile([128, 2 * QB], F32, tag="big", bufs=3,
                             name=name)

        def emit_body(a2a_local: bool):
            # ================= QKV projection (transposed) =================
            vt_tiles = {}

            def qk_chain_unit(b, tsr, pp):
                """One [128,1024] psum tile = two 512-row chains (rows
                2pp*QB..), contraction inner. Drained by one DVE bias add."""
                def run():
                    if tsr == 2 and b not in vt_tiles:
                        vt_tiles[b] = work.tile([128, S], BF16, tag="vt",
                                                name=f"vt{b}")
                    ps = big_ps(0, "ps_qkv")
                    for cc in range(CCH):
                        for rc in (2 * pp, 2 * pp + 1):
                            r0 = b * S + rc * QB
                            half = (rc % 2) * QB
                            nc.tensor.matmul(
                                ps[:, half:half + QB],
                                wit_sb[:, cc, tsr * HD:(tsr + 1) * HD],
                                xt_sb[:, cc, r0:r0 + QB],
                                start=(cc == 0), stop=(cc == CCH - 1),
                            )
                    dst = (qt_sb[b] if tsr == 0 else
                           kt_sb[b] if tsr == 1 else vt_tiles[b])
                    nc.vector.tensor_scalar(
                        dst[:, pp * 2 * QB:(pp + 1) * 2 * QB], ps[:],
                        bias_sb[:, tsr:tsr + 1], None, ALU.add)
                return run

            def vtr_unit(b, rb0, psum_tag):
                """Transpose vT blocks rb0..rb0+3 -> v_sb [k rows, dims]."""
                def run():
                    for rb in range(rb0, rb0 + 4):
                        if psum_tag == "big":
                            pst = psum.tile([128, 128], BF16, tag="big",
                                            bufs=3, name="ps_tr")
                        else:
                            pst = psum.tile([128, 128], BF16,
                                            tag=f"o{rb % 2}",
                                            bufs=1, name="ps_tr")
                        nc.tensor.transpose(
                            pst[:],
                            vt_tiles[b][:, rb * 128:(rb + 1) * 128],
                            identity[:])
                        g = b * NKB + rb
                        for h in range(HPC):
                            nc.vector.tensor_copy(
                                v_sb[h][:, g, 0:DH],
                                pst[:, h * DH:(h + 1) * DH])
                return run

            def qkv_units(b, psum_tag="o"):
                us = []
                for tsr in range(3):
                    for pp in range(2):
                        us.append(qk_chain_unit(b, tsr, pp))
                for rb0 in range(0, S // 128, 4):
                    us.append(vtr_unit(b, rb0, psum_tag))
                return us

            for u in qkv_units(0):
                u()
            if parts == "qkv":
                for u in qkv_units(1):
                    u()
                return
            # batch 1's projection is deferred: its units are fed one at a
            # time into batch 0's (ACT-bound) attention stream so the PE
            # fills its exp-wait and normalize-tail gaps with useful work
            pending = qkv_units(1, psum_tag="big")

            def feed(n=1):
                for _ in range(n):
                    if pending:
                        pending.pop(0)()

            # ======================= attention =============================
            # scores run one k-block ahead of PV so PE never waits on exp
            for b in range(B):
                if b == 1:
                    while pending:
                        feed()
                for qb in range(NQB):
                    nkb = 4 * (qb + 1) if causal else NKB
                    q0 = qb * QB
                    ps_o = [psum.tile([DH + 1, QB], F32, tag=f"o{h}", bufs=1,
                                      name=f"ps_o{h}")
                            for h in range(HPC)]

                    def scores_pair(p):
                        """Two k-blocks (2p, 2p+1) -> one [128,1024] psum per
                        head, a single exp. Causal masking is folded into the
                        scores additively: for diagonal sub-blocks one extra
                        matmul (identity stationary x additive-mask pattern)
                        accumulates -1e9 onto masked positions, so exp yields
                        exact zeros with no DVE involvement. MMs alternate
                        heads so consecutive matmuls land on different PE
                        row-groups and overlap in the array."""
                        pss = [big_ps(h, f"ps_s{h}") for h in range(HPC)]
                        for half in range(2):
                            kb = 2 * p + half
                            rel = kb - 4 * qb
                            diag = (causal and "nodiag" not in parts
                                    and 0 <= rel <= 3)
                            for h in range(HPC):
                                nc.tensor.matmul(
                                    pss[h][:, half * QB:(half + 1) * QB],
                                    kt_sb[b][h * DH:(h + 1) * DH,
                                             kb * KB:(kb + 1) * KB],
                                    qt_sb[b][h * DH:(h + 1) * DH, q0:q0 + QB],
                                    start=True, stop=not diag,
                                )
                            if diag:
                                c0 = half * QB + rel * KB
                                for h in range(HPC):
                                    nc.tensor.matmul(
                                        pss[h][:, c0:c0 + KB],
                                        identity[:], mask_sb[:],
                                        start=False, stop=True,
                                        skip_group_check=True,
                                    )
                        es = []
                        for h in range(HPC):
                            ps_s = pss[h]
                            e = epool.tile([128, 2 * QB], BF16, tag="expT",
                                           name="expT")
                            if "peonly" in parts:
                                es.append(None)
                                continue
                            if "noexp" in parts:
                                # tiny write so the tile is allocated; PV
                                # reads garbage (timing probe only)
                                nc.gpsimd.memset(e[:, 0:1], 0.0)
                                es.append(e)
                                continue
                            t = 2 * p - 4 * qb if causal else -1
                            if "nodiag" in parts:
                                t = -1
                            if causal and t >= 0:
                                # halves are diagonal blocks t and t+1.
                                # Fully-masked prefixes of each half are
                                # never exp'd or zeroed: the PV matmuls
                                # column-slice past them instead. Triangle
                                # masking already happened additively in the
                                # scores psum.
                                c0 = t * KB
                                nc.scalar.activation(
                                    e[:, c0:2 * QB], ps_s[:, c0:2 * QB],
                                    AF.Exp, scale=1.0 / 8.0)
                            else:
                                nc.scalar.activation(e[:], ps_s[:], AF.Exp,
                                                     scale=1.0 / 8.0)
                            es.append(e)
                        return es

                    def pv_pair(p, es):
                        for h in range(HPC):
                            for half in range(2):
                                kb = 2 * p + half
                                # causal: queries before co see only masked
                                # keys in this block -> skip those columns
                                co = (max(0, (2 * p + half - 4 * qb) * KB)
                                      if (causal and "nodiag" not in parts)
                                      else 0)
                                if es[h] is None:
                                    rhs = qt_sb[b][:, q0:q0 + QB]
                                    co = 0
                                else:
                                    rhs = es[h][:, half * QB + co:
                                                (half + 1) * QB]
                                nc.tensor.matmul(
                                    ps_o[h][:, co:QB],
                                    v_sb[h][:, b * NKB + kb, :],
                                    rhs,
                                    start=(kb == 0), stop=(kb == nkb - 1),
                                )

                    npair = nkb // 2
                    es_prev = scores_pair(0)
                    for p in range(1, npair):
                        es = scores_pair(p)
                        pv_pair(p - 1, es_prev)
                        es_prev = es
                    pv_pair(npair - 1, es_prev)

                    at = work.tile([128, QB], BF16, tag="attnT", name="attnT")
                    if "nonorm" in parts:
                        nc.vector.tensor_copy(at[0:DH, :], ps_o[0][0:DH, :])
                        nc.vector.tensor_copy(at[DH:2 * DH, :],
                                              ps_o[1][0:DH, :])
                    else:
                        # evacuate the PV accumulators (values + denominator
                        # row) to SBUF immediately so the psum banks free up
                        # for the next strip's PV; then the broadcast and the
                        # normalize multiplies run on the otherwise-idle
                        # GPSIMD engine so the DVE FIFO stays clear for the
                        # next strip's triangle-mask muls (which gate PV).
                        last = (b == B - 1 and qb == NQB - 1)
                        rcr = work.tile([1, 2 * QB], F32, tag="rcr",
                                        name="rcr")
                        if last:
                            # nothing competes for these psum banks after the
                            # final strip: skip the evacuation copies and
                            # normalize straight out of psum (shorter chain
                            # before the exchange + output projection)
                            for h in range(HPC):
                                nc.vector.reciprocal(
                                    rcr[0:1, h * QB:(h + 1) * QB],
                                    ps_o[h][DH:DH + 1, :])
                            vals = [ps_o[h][0:DH, :] for h in range(HPC)]
                        else:
                            pvs = work.tile([128, QB], F32, tag="pvs",
                                            name="pvs")
                            rc = work.tile([1, 2 * QB], F32, tag="rc",
                                           name="rc")
                            for h in range(HPC):
                                nc.vector.tensor_copy(
                                    pvs[h * DH:(h + 1) * DH, :],
                                    ps_o[h][0:DH, :])
                                nc.vector.tensor_copy(
                                    rc[0:1, h * QB:(h + 1) * QB],
                                    ps_o[h][DH:DH + 1, :])
                            nc.vector.reciprocal(rcr[:], rc[:])
                            vals = [pvs[h * DH:(h + 1) * DH, :]
                                    for h in range(HPC)]
                        rpb = work.tile([128, 2 * QB], F32, tag="rpb",
                                        name="rpb")
                        nc.gpsimd.partition_broadcast(rpb[:], rcr[0:1, :])
                        for h in range(HPC):
                            nc.vector.tensor_mul(
                                at[h * DH:(h + 1) * DH, :],
                                vals[h],
                                rpb[h * DH:(h + 1) * DH,
                                    h * QB:(h + 1) * QB])
                    j = b * NQB + qb
                    nc.sync.dma_start(a2a_in[j], at[:])
                    if a2a_local:
                        # timing stand-in for the exchange, overlapped with
                        # the rest of attention (collectives can't sit in
                        # control flow)
                        nc.sync.dma_start(a2a_out[j], a2a_in[j])
                        nc.sync.dma_start(ao_sb[:, j, :], a2a_out[j])
                    if b == 0:
                        # fill the strip-tail normalize window with deferred
                        # batch-1 projection work (exactly one unit: a second
                        # one overruns the idle window and stalls the next
                        # strip's exp stream)
                        feed()

            # ================= AllToAll + output projection ================
            if parts == "qkv+att":
                return
            if not a2a_local:
                nc.gpsimd.collective_compute(
                    "AllToAll", ALU.bypass,
                    replica_groups=[list(range(NCORES))],
                    ins=[a2a_in[:]], outs=[a2a_out[:]],
                )
                for j in range(NCORES):
                    nc.sync.dma_start(ao_sb[:, j, :], a2a_out[j])
            # 4 waves of 2 output chains each through the 3 rotating big
            # psum slots. The first 3 waves run their j=0..6 accumulation
            # matmuls up front (those shards arrived before the last strip)
            # and defer the j=7 matmuls to the end, so the PE keeps working
            # under the last strip's normalize/exchange tail instead of
            # head-of-line blocking on ao_sb[7].
            def opj_mm(ps, w, j, start, stop):
                for half in range(2):
                    ob = 2 * w + half
                    nc.tensor.matmul(
                        ps[:, half * RPC:(half + 1) * RPC],
                        wot_sb[:, j, ob * 128:(ob + 1) * 128],
                        ao_sb[:, j, :],
                        start=start, stop=stop,
                    )

            def opj_drain(ps, w):
                os = work.tile([128, 2 * RPC], F32, tag="os", name="os")
                for half in range(2):
                    ob = 2 * w + half
                    nc.vector.tensor_scalar(
                        os[:, half * RPC:(half + 1) * RPC],
                        ps[:, half * RPC:(half + 1) * RPC],
                        bo_sb[:, ob:ob + 1], None, ALU.add)
                    nc.sync.dma_start(out_t[ob * 128:(ob + 1) * 128, :],
                                      os[:, half * RPC:(half + 1) * RPC])

            waves = []
            for w in range(3):
                ps = big_ps(w, f"ps_outp{w}")
                for j in range(NCORES - 1):
                    opj_mm(ps, w, j, start=(j == 0), stop=False)
                waves.append(ps)
            for w in range(3):
                opj_mm(waves[w], w, NCORES - 1, start=False, stop=True)
                opj_drain(waves[w], w)
            ps = big_ps(3, "ps_outp3")
            for j in range(NCORES):
                opj_mm(ps, 3, j, start=(j == 0), stop=(j == NCORES - 1))
            opj_drain(ps, 3)

        if loop_n:
            with tc.For_i(0, loop_n, 1,
                          hint_engines=(mybir.EngineType.PE,
                                        mybir.EngineType.DVE,
                                        mybir.EngineType.Activation)):
                emit_body(a2a_local=True)
            dsb = persist.tile([1, 16], F32)
            nc.vector.memset(dsb[:], 0.0)
            nc.sync.dma_start(dummy[:], dsb[:])
        else:
            for _ in range(repeat):
                emit_body(a2a_local=a2a_local)

    nc.compile()
    return nc


def _build_a2a_bench(k: int):
    """k back-to-back AllToAlls on the kernel's exchange buffer size."""
    nc = bacc.Bacc("TRN2", target_bir_lowering=False, debug=False,
                   num_devices=NCORES)
    src = nc.dram_tensor("src", [NCORES, HD, RPC], BF16,
                         kind="ExternalInput").ap()
    dst = nc.dram_tensor("dst", [1, 16], F32, kind="ExternalOutput").ap()
    with tile.TileContext(nc) as tc, ExitStack() as octx:
        dram = octx.enter_context(tc.tile_pool(name="dram", bufs=1,
                                               space="DRAM"))
        pool = octx.enter_context(tc.tile_pool(name="sb", bufs=1))
        a = dram.tile([NCORES, HD, RPC], BF16)
        bb = dram.tile([NCORES, HD, RPC], BF16)
        nc.sync.dma_start(a[:], src[:])
        bufs = [a, bb]
        for i in range(k):
            nc.gpsimd.collective_compute(
                "AllToAll", ALU.bypass,
                replica_groups=[list(range(NCORES))],
                ins=[bufs[i % 2][:]], outs=[bufs[(i + 1) % 2][:]],
            )
        dsb = pool.tile([1, 16], F32)
        nc.vector.memset(dsb[:], 0.0)
        nc.sync.dma_start(dst[:], dsb[:])
    nc.compile()
    return nc


def _make_runner(nc):
    """Jitted 8-core SPMD executor for a compiled Bass module."""
    install_neuronx_cc_hook()
    partition_name = nc.partition_id_tensor.name if nc.partition_id_tensor else None
    in_names, out_names, out_avals = [], [], []
    for alloc in nc.m.functions[0].allocations:
        if not isinstance(alloc, mybir.MemoryLocationSet):
            continue
        name = alloc.memorylocations[0].name
        if alloc.kind == "ExternalInput":
            if name != partition_name:
                in_names.append(name)
        elif alloc.kind == "ExternalOutput":
            out_names.append(name)
            out_avals.append(jax.core.ShapedArray(
                tuple(alloc.tensor_shape), mybir.dt.np(alloc.dtype)))
    n_params = len(in_names)
    n_outs = len(out_avals)
    all_in_names = list(in_names) + list(out_names)
    if partition_name is not None:
        all_in_names.append(partition_name)
    donate = tuple(range(n_params, n_params + n_outs))

    def _body(*args):
        operands = list(args)
        if partition_name is not None:
            operands.append(partition_id_tensor())
        return tuple(_bass_exec_p.bind(
            *operands,
            out_avals=tuple(out_avals),
            in_names=tuple(all_in_names),
            out_names=tuple(out_names),
            lowering_input_output_aliases=(),
            sim_require_finite=True,
            sim_require_nnan=True,
            nc=nc,
        ))

    devices = jax.devices()[:NCORES]
    mesh = Mesh(np.asarray(devices), ("core",))
    sharded = jax.jit(
        shard_map(_body, mesh=mesh,
                  in_specs=(PartitionSpec("core"),) * (n_params + n_outs),
                  out_specs=(PartitionSpec("core"),) * n_outs,
                  check_rep=False),
        donate_argnums=donate, keep_unused=True)

    zero_shapes = [a.shape for a in out_avals]
    zero_dtypes = [a.dtype for a in out_avals]

    def _zeros():
        return [np.zeros((NCORES * s[0], *s[1:]), d)
                for s, d in zip(zero_shapes, zero_dtypes)]

    def prepare(in_maps):
        """Concatenate per-core inputs and stage them on device once."""
        return [
            jax.device_put(np.concatenate(
                [np.asarray(m[name]) for m in in_maps], axis=0))
            for name in in_names
        ]

    def run_prepared(handles, as_numpy=True):
        out_arrs = sharded(*handles, *_zeros())
        if not as_numpy:
            jax.block_until_ready(out_arrs)
            return out_arrs
        return [
            {name: np.asarray(out_arrs[i]).reshape(NCORES, *zero_shapes[i])[c]
             for i, name in enumerate(out_names)}
            for c in range(NCORES)
        ]

    def run(in_maps):
        return run_prepared(prepare(in_maps))

    run.prepare = prepare
    run.run_prepared = run_prepared
    return run


def _shard_inputs(x, Wi, bi, Wo, bo, causal):
    """Host-side slicing/layout prep -> per-core input maps."""
    bf = ml_dtypes.bfloat16
    x = np.asarray(x, np.float32)
    Wi = np.asarray(Wi, np.float32)
    bi = np.asarray(bi, np.float32)
    Wo = np.asarray(Wo, np.float32)
    bo = np.asarray(bo, np.float32)

    xt = np.ascontiguousarray(x.reshape(R, D).T).astype(bf)       # (D, R)
    wot = np.ascontiguousarray(Wo.T).astype(bf)                   # (D, D)

    if causal:
        # additive mask pattern, accumulated onto diagonal score sub-blocks
        # via matmul(identity, m): scores[k, q] += m[k, q]; masked (k > q)
        # positions get -1e9 so exp underflows to exactly 0
        i = np.arange(KB)[:, None]
        j = np.arange(KB)[None, :]
        m = np.where(j >= i, 0.0, -1e9).astype(bf)
    else:
        m = np.zeros((KB, KB), bf)

    in_maps = []
    for c in range(NCORES):
        rows = np.concatenate([
            np.arange(c * HD, (c + 1) * HD),
            D + np.arange(c * HD, (c + 1) * HD),
            2 * D + np.arange(c * HD, (c + 1) * HD),
        ])
        wit_c = np.ascontiguousarray(Wi[rows].T).astype(bf)       # (D, 384)
        bi_c = np.ascontiguousarray(bi[rows]).astype(np.float32)  # (384,)
        in_maps.append({
            "xt": xt, "wit": wit_c, "bi_s": bi_c,
            "wot": wot, "bo_f": bo, "masks": m,
        })
    return in_maps


_CACHE = {}


def _get_runner(causal, repeat=1):
    key = (causal, repeat)
    if key not in _CACHE:
        nc = _build(causal, repeat)
        _CACHE[key] = _make_runner(nc)
    return _CACHE[key]


def kernel(x, Wi, bi, Wo, bo, causal_mask):
    causal = bool(int(np.asarray(causal_mask)))
    run = _get_runner(causal)
    in_maps = _shard_inputs(x, Wi, bi, Wo, bo, causal)
    res = run(in_maps)
    # res[c]["out_t"]: (D, RPC) fp32 = transposed rows [c*RPC, (c+1)*RPC)
    full = np.concatenate([res[c]["out_t"].T for c in range(NCORES)], axis=0)
    return np.ascontiguousarray(full.reshape(B, S, D).astype(np.float32))



# revision 48
# speedup vs baseline: 1.0307x; 1.0307x over previous
"""Multi-head causal attention (B=2, S=2048, D=1024, H=16) on 8 Trainium2 cores.

Sharding: tensor-parallel over heads. Core c computes QKV projection, causal
attention and softmax for heads {2c, 2c+1} over both batches, then an AllToAll
redistributes the attention output so core c owns rows [512c, 512c+512) of the
flattened (B*S, D) activation; each core applies the full output projection to
its row slice. Host code only slices/transposes inputs and concatenates the
per-core output slices.

All matmuls run in bf16 with fp32 PSUM accumulation. The pipeline works in
transposed layout ([dim, seq]) so that softmax reduces over the PSUM partition
axis via a ones-column folded into the PV matmul, and the attention output
lands directly in the layout the output projection consumes.

Key scheduling decisions (measured on HW via same-process A/B):
- Causal masking is additive, done on the PE (identity x mask-pattern matmul
  accumulated onto diagonal score blocks before exp), so the DVE is not on
  the scores->exp->PV critical path and exp underflows masked entries to 0.
- PV matmuls column-slice past fully-masked query prefixes (no memsets).
- The per-strip softmax normalization evacuates the PV psum accumulators to
  SBUF with two DVE copies first, freeing the banks for the next strip; the
  recip/gpsimd-broadcast/mul chain then runs off the critical path.
- Batch 1's QKV chains are deferred into closures fed into batch 0's
  (ACT-bound) attention stream at strip tails, filling PE idle windows.
- Output projection runs ob-outer/j-inner in 4 two-chain waves through the
  rotating score psum slots.
"""
import numpy as np
from contextlib import ExitStack

import jax
import ml_dtypes

import concourse.bass as bass
import concourse.tile as tile
from concourse import bacc, mybir
from concourse.bass2jax import (
    _bass_exec_p,
    install_neuronx_cc_hook,
    partition_id_tensor,
)
from jax.sharding import Mesh, PartitionSpec
from jax.experimental.shard_map import shard_map

B, S, D, H = 2, 2048, 1024, 16
DH = D // H            # 64
NCORES = 8
HPC = H // NCORES      # heads per core = 2
HD = HPC * DH          # head dims per core = 128
R = B * S              # flattened rows = 4096
RPC = R // NCORES      # rows per core after AllToAll = 512
QB = 512               # query block (also the AllToAll shard size)
KB = 128               # key block
NQB = S // QB          # 4 query blocks per batch
NKB = S // KB          # 16 key blocks per batch
CCH = D // 128         # contraction chunks for D-wide matmuls = 8

BF16 = mybir.dt.bfloat16
F32 = mybir.dt.float32
AF = mybir.ActivationFunctionType
ALU = mybir.AluOpType


def _build(causal: bool, repeat: int = 1, loop_n: int = 0,
           a2a_local: bool = False, parts: str = "full"):
    """Emit the SPMD Bass program (identical on all 8 cores).

    loop_n > 0 builds a timing variant: the whole per-iteration body runs
    inside a hardware For_i loop and the AllToAll is replaced by a local DMA
    copy (collectives cannot sit inside control flow), with the real output
    replaced by a tiny dummy (so the timing loop's donated output buffers are
    negligible to transfer). Used only to measure per-iteration device time.
    """
    timing = loop_n > 0
    nc = bacc.Bacc("TRN2", target_bir_lowering=False, debug=False,
                   num_devices=NCORES)

    xt = nc.dram_tensor("xt", [D, R], BF16, kind="ExternalInput").ap()
    wit = nc.dram_tensor("wit", [D, 3 * HD], BF16, kind="ExternalInput").ap()
    bi_s = nc.dram_tensor("bi_s", [3 * HD], F32, kind="ExternalInput").ap()
    wot = nc.dram_tensor("wot", [D, D], BF16, kind="ExternalInput").ap()
    bo_f = nc.dram_tensor("bo_f", [D], F32, kind="ExternalInput").ap()
    masks = nc.dram_tensor("masks", [KB, KB], BF16, kind="ExternalInput").ap()
    if timing:
        out_t = nc.dram_tensor("out_scratch", [D, RPC], F32).ap()
        dummy = nc.dram_tensor("tiny_out", [1, 16], F32, kind="ExternalOutput").ap()
    else:
        out_t = nc.dram_tensor("out_t", [D, RPC], F32, kind="ExternalOutput").ap()

    with tile.TileContext(nc) as tc, ExitStack() as octx:
        persist = octx.enter_context(tc.tile_pool(name="persist", bufs=1))
        dram = octx.enter_context(tc.tile_pool(name="dram", bufs=1, space="DRAM"))

        # ---- persistent SBUF state (x chunks queued right after wit: the
        # QKV matmuls need them first; wot/bo only matter at the end) ----
        wit_sb = persist.tile([128, CCH, 3 * HD], BF16)
        nc.sync.dma_start(wit_sb[:], wit.rearrange("(cc p) n -> p cc n", p=128))
        bias_sb = persist.tile([128, 3], F32)
        nc.sync.dma_start(bias_sb[:], bi_s.rearrange("(t p) -> p t", p=128))
        xt_pool = octx.enter_context(tc.tile_pool(name="xt_pool", bufs=1))
        xt_sb = xt_pool.tile([128, CCH, R], BF16)
        xt_r = xt.rearrange("(cc p) r -> p cc r", p=128)
        for cc in range(CCH):
            nc.sync.dma_start(xt_sb[:, cc, :], xt_r[:, cc, :])
        wot_sb = persist.tile([128, CCH, D], BF16)
        nc.sync.dma_start(wot_sb[:], wot.rearrange("(cc p) o -> p cc o", p=128))
        bo_sb = persist.tile([128, CCH], F32)
        nc.sync.dma_start(bo_sb[:], bo_f.rearrange("(oc p) -> p oc", p=128))
        # one [128,128] triangle: mask[i, j] = (j >= i), same for every
        # diagonal sub-block once the exp is column-sliced
        mask_sb = persist.tile([128, KB], BF16)
        if causal:
            nc.sync.dma_start(mask_sb[:], masks[:])

        identity = persist.tile([128, 128], BF16)
        from concourse.masks import make_identity
        make_identity(nc, identity[:])
        ones_row = persist.tile([1, 128], BF16)
        nc.vector.memset(ones_row[:], 1.0)

        # qT/kT: [head-dims (2 heads x 64), S] per batch; v: [k rows, 65] blocks
        qt_sb = [persist.tile([128, S], BF16, name=f"qt{b}") for b in range(B)]
        kt_sb = [persist.tile([128, S], BF16, name=f"kt{b}") for b in range(B)]
        # v_sb[h][:, g, 0:64] = v rows for global k-block g; col 64 = 1.0
        v_sb = [persist.tile([128, B * NKB, DH + 1], BF16, name=f"v{h}")
                for h in range(HPC)]
        for h in range(HPC):
            nc.vector.memset(v_sb[h][:, :, DH:DH + 1], 1.0)

        a2a_in = dram.tile([NCORES, HD, RPC], BF16)
        a2a_out = dram.tile([NCORES, HD, RPC], BF16)
        ao_sb = persist.tile([128, NCORES, RPC], BF16, name="ao_sb")

        # one PSUM pool for all phases; tags share slots so cross-phase reuse
        # only waits on the previous user of one slot, not a whole phase.
        # banks: big0(2) + big1(2) + tr(2) + o0(1) + o1(1) = 8
        psum = octx.enter_context(tc.tile_pool(name="psum", bufs=1,
                                               space="PSUM"))
        work = octx.enter_context(tc.tile_pool(name="work", bufs=3))
        epool = octx.enter_context(tc.tile_pool(name="epool", bufs=6))

        def big_ps(i, name):
            # [128, 1024] = 2 PSUM banks; halves are used as two 512-wide
            # chains / score blocks so ACT+DVE consumers see one wide AP.
            # One tag, 3 rotating slots (6 banks): the head-interleaved score
            # stream gets effective double-buffering without per-head tags.
            return psum.tile([128, 2 * QB], F32, tag="big", bufs=3,
                             name=name)

        def emit_body(a2a_local: bool):
            # ================= QKV projection (transposed) =================
            vt_tiles = {}

            def qk_chain_unit(b, tsr, pp):
                """One [128,1024] psum tile = two 512-row chains (rows
                2pp*QB..), contraction inner. Drained by one DVE bias add."""
                def run():
                    if tsr == 2 and b not in vt_tiles:
                        vt_tiles[b] = work.tile([128, S], BF16, tag="vt",
                                                name=f"vt{b}")
                    ps = big_ps(0, "ps_qkv")
                    for cc in range(CCH):
                        for rc in (2 * pp, 2 * pp + 1):
                            r0 = b * S + rc * QB
                            half = (rc % 2) * QB
                            nc.tensor.matmul(
                                ps[:, half:half + QB],
                                wit_sb[:, cc, tsr * HD:(tsr + 1) * HD],
                                xt_sb[:, cc, r0:r0 + QB],
                                start=(cc == 0), stop=(cc == CCH - 1),
                            )
                    dst = (qt_sb[b] if tsr == 0 else
                           kt_sb[b] if tsr == 1 else vt_tiles[b])
                    nc.vector.tensor_scalar(
                        dst[:, pp * 2 * QB:(pp + 1) * 2 * QB], ps[:],
                        bias_sb[:, tsr:tsr + 1], None, ALU.add)
                return run

            def vtr_unit(b, rb0, psum_tag):
                """Transpose vT blocks rb0..rb0+3 -> v_sb [k rows, dims]."""
                def run():
                    for rb in range(rb0, rb0 + 4):
                        if psum_tag == "big":
                            pst = psum.tile([128, 128], BF16, tag="big",
                                            bufs=3, name="ps_tr")
                        else:
                            pst = psum.tile([128, 128], BF16,
                                            tag=f"o{rb % 2}",
                                            bufs=1, name="ps_tr")
                        nc.tensor.transpose(
                            pst[:],
                            vt_tiles[b][:, rb * 128:(rb + 1) * 128],
                            identity[:])
                        g = b * NKB + rb
                        for h in range(HPC):
                            nc.vector.tensor_copy(
                                v_sb[h][:, g, 0:DH],
                                pst[:, h * DH:(h + 1) * DH])
                return run

            def qkv_units(b, psum_tag="o"):
                us = []
                for tsr in range(3):
                    for pp in range(2):
                        us.append(qk_chain_unit(b, tsr, pp))
                for rb0 in range(0, S // 128, 4):
                    us.append(vtr_unit(b, rb0, psum_tag))
                return us

            for u in qkv_units(0):
                u()
            if parts == "qkv":
                for u in qkv_units(1):
                    u()
                return
            # batch 1's projection is deferred: its units are fed one at a
            # time into batch 0's (ACT-bound) attention stream so the PE
            # fills its exp-wait and normalize-tail gaps with useful work
            pending = qkv_units(1, psum_tag="big")

            def feed(n=1):
                for _ in range(n):
                    if pending:
                        pending.pop(0)()

            # ======================= attention =============================
            # scores run one k-block ahead of PV so PE never waits on exp
            for b in range(B):
                if b == 1:
                    while pending:
                        feed()
                for qb in range(NQB):
                    nkb = 4 * (qb + 1) if causal else NKB
                    q0 = qb * QB
                    ps_o = [psum.tile([DH + 1, QB], F32, tag=f"o{h}", bufs=1,
                                      name=f"ps_o{h}")
                            for h in range(HPC)]

                    def scores_pair(p):
                        """Two k-blocks (2p, 2p+1) -> one [128,1024] psum per
                        head, a single exp. Causal masking is folded into the
                        scores additively: for diagonal sub-blocks one extra
                        matmul (identity stationary x additive-mask pattern)
                        accumulates -1e9 onto masked positions, so exp yields
                        exact zeros with no DVE involvement. MMs alternate
                        heads so consecutive matmuls land on different PE
                        row-groups and overlap in the array."""
                        pss = [big_ps(h, f"ps_s{h}") for h in range(HPC)]
                        for half in range(2):
                            kb = 2 * p + half
                            rel = kb - 4 * qb
                            diag = (causal and "nodiag" not in parts
                                    and 0 <= rel <= 3)
                            for h in range(HPC):
                                nc.tensor.matmul(
                                    pss[h][:, half * QB:(half + 1) * QB],
                                    kt_sb[b][h * DH:(h + 1) * DH,
                                             kb * KB:(kb + 1) * KB],
                                    qt_sb[b][h * DH:(h + 1) * DH, q0:q0 + QB],
                                    start=True, stop=not diag,
                                )
                            if diag:
                                c0 = half * QB + rel * KB
                                for h in range(HPC):
                                    nc.tensor.matmul(
                                        pss[h][:, c0:c0 + KB],
                                        identity[:], mask_sb[:],
                                        start=False, stop=True,
                                        skip_group_check=True,
                                    )
                        es = []
                        for h in range(HPC):
                            ps_s = pss[h]
                            e = epool.tile([128, 2 * QB], BF16, tag="expT",
                                           name="expT")
                            if "peonly" in parts:
                                es.append(None)
                                continue
                            if "noexp" in parts:
                                # tiny write so the tile is allocated; PV
                                # reads garbage (timing probe only)
                                nc.gpsimd.memset(e[:, 0:1], 0.0)
                                es.append(e)
                                continue
                            t = 2 * p - 4 * qb if causal else -1
                            if "nodiag" in parts:
                                t = -1
                            if causal and t >= 0:
                                # halves are diagonal blocks t and t+1.
                                # Fully-masked prefixes of each half are
                                # never exp'd or zeroed: the PV matmuls
                                # column-slice past them instead. Triangle
                                # masking already happened additively in the
                                # scores psum.
                                c0 = t * KB
                                nc.scalar.activation(
                                    e[:, c0:2 * QB], ps_s[:, c0:2 * QB],
                                    AF.Exp, scale=1.0 / 8.0)
                            else:
                                nc.scalar.activation(e[:], ps_s[:], AF.Exp,
                                                     scale=1.0 / 8.0)
                            es.append(e)
                        return es

                    def pv_pair(p, es):
                        for h in range(HPC):
                            for half in range(2):
                                kb = 2 * p + half
                                # causal: queries before co see only masked
                                # keys in this block -> skip those columns
                                co = (max(0, (2 * p + half - 4 * qb) * KB)
                                      if (causal and "nodiag" not in parts)
                                      else 0)
                                if es[h] is None:
                                    rhs = qt_sb[b][:, q0:q0 + QB]
                                    co = 0
                                else:
                                    rhs = es[h][:, half * QB + co:
                                                (half + 1) * QB]
                                nc.tensor.matmul(
                                    ps_o[h][:, co:QB],
                                    v_sb[h][:, b * NKB + kb, :],
                                    rhs,
                                    start=(kb == 0), stop=(kb == nkb - 1),
                                )

                    npair = nkb // 2
                    es_prev = scores_pair(0)
                    for p in range(1, npair):
                        es = scores_pair(p)
                        pv_pair(p - 1, es_prev)
                        es_prev = es
                    pv_pair(npair - 1, es_prev)

                    at = work.tile([128, QB], BF16, tag="attnT", name="attnT")
                    if "nonorm" in parts:
                        nc.vector.tensor_copy(at[0:DH, :], ps_o[0][0:DH, :])
                        nc.vector.tensor_copy(at[DH:2 * DH, :],
                                              ps_o[1][0:DH, :])
                    else:
                        # evacuate the PV accumulators (values + denominator
                        # row) to SBUF immediately so the psum banks free up
                        # for the next strip's PV; then the broadcast and the
                        # normalize multiplies run on the otherwise-idle
                        # GPSIMD engine so the DVE FIFO stays clear for the
                        # next strip's triangle-mask muls (which gate PV).
                        last = (b == B - 1 and qb == NQB - 1)
                        rcr = work.tile([1, 2 * QB], F32, tag="rcr",
                                        name="rcr")
                        if last:
                            # nothing competes for these psum banks after the
                            # final strip: skip the evacuation copies and
                            # normalize straight out of psum (shorter chain
                            # before the exchange + output projection)
                            for h in range(HPC):
                                nc.vector.reciprocal(
                                    rcr[0:1, h * QB:(h + 1) * QB],
                                    ps_o[h][DH:DH + 1, :])
                            vals = [ps_o[h][0:DH, :] for h in range(HPC)]
                        else:
                            pvs = work.tile([128, QB], F32, tag="pvs",
                                            name="pvs")
                            rc = work.tile([1, 2 * QB], F32, tag="rc",
                                           name="rc")
                            for h in range(HPC):
                                nc.vector.tensor_copy(
                                    pvs[h * DH:(h + 1) * DH, :],
                                    ps_o[h][0:DH, :])
                                nc.vector.tensor_copy(
                                    rc[0:1, h * QB:(h + 1) * QB],
                                    ps_o[h][DH:DH + 1, :])
                            nc.vector.reciprocal(rcr[:], rc[:])
                            vals = [pvs[h * DH:(h + 1) * DH, :]
                                    for h in range(HPC)]
                        rpb = work.tile([128, 2 * QB], F32, tag="rpb",
                                        name="rpb")
                        nc.gpsimd.partition_broadcast(rpb[:], rcr[0:1, :])
                        for h in range(HPC):
                            nc.vector.tensor_mul(
                                at[h * DH:(h + 1) * DH, :],
                                vals[h],
                                rpb[h * DH:(h + 1) * DH,
                                    h * QB:(h + 1) * QB])
                    j = b * NQB + qb
                    nc.sync.dma_start(a2a_in[j], at[:])
                    if a2a_local:
                        # timing stand-in for the exchange (collectives can't
                        # sit in control flow): the real kernel's data path is
                        # at -> a2a_in -> [AllToAll, charged separately by the
                        # harness] -> ao_sb, so the local build mirrors just
                        # the two endpoint DMAs
                        nc.sync.dma_start(ao_sb[:, j, :], a2a_in[j])
                    if b == 0:
                        # fill the strip-tail normalize window with deferred
                        # batch-1 projection work (exactly one unit: a second
                        # one overruns the idle window and stalls the next
                        # strip's exp stream)
                        feed()

            # ================= AllToAll + output projection ================
            if parts == "qkv+att":
                return
            if not a2a_local:
                nc.gpsimd.collective_compute(
                    "AllToAll", ALU.bypass,
                    replica_groups=[list(range(NCORES))],
                    ins=[a2a_in[:]], outs=[a2a_out[:]],
                )
                for j in range(NCORES):
                    nc.sync.dma_start(ao_sb[:, j, :], a2a_out[j])
            # 4 waves of 2 output chains each through the 3 rotating big
            # psum slots. The first 3 waves run their j=0..6 accumulation
            # matmuls up front (those shards arrived before the last strip)
            # and defer the j=7 matmuls to the end, so the PE keeps working
            # under the last strip's normalize/exchange tail instead of
            # head-of-line blocking on ao_sb[7].
            def opj_mm(ps, w, j, start, stop):
                for half in range(2):
                    ob = 2 * w + half
                    nc.tensor.matmul(
                        ps[:, half * RPC:(half + 1) * RPC],
                        wot_sb[:, j, ob * 128:(ob + 1) * 128],
                        ao_sb[:, j, :],
                        start=start, stop=stop,
                    )

            def opj_drain(ps, w):
                os = work.tile([128, 2 * RPC], F32, tag="os", name="os")
                for half in range(2):
                    ob = 2 * w + half
                    nc.vector.tensor_scalar(
                        os[:, half * RPC:(half + 1) * RPC],
                        ps[:, half * RPC:(half + 1) * RPC],
                        bo_sb[:, ob:ob + 1], None, ALU.add)
                    nc.sync.dma_start(out_t[ob * 128:(ob + 1) * 128, :],
                                      os[:, half * RPC:(half + 1) * RPC])

            waves = []
            for w in range(3):
                ps = big_ps(w, f"ps_outp{w}")
                for j in range(NCORES - 1):
                    opj_mm(ps, w, j, start=(j == 0), stop=False)
                waves.append(ps)
            for w in range(3):
                opj_mm(waves[w], w, NCORES - 1, start=False, stop=True)
                opj_drain(waves[w], w)
            ps = big_ps(3, "ps_outp3")
            for j in range(NCORES):
                opj_mm(ps, 3, j, start=(j == 0), stop=(j == NCORES - 1))
            opj_drain(ps, 3)

        if loop_n:
            with tc.For_i(0, loop_n, 1,
                          hint_engines=(mybir.EngineType.PE,
                                        mybir.EngineType.DVE,
                                        mybir.EngineType.Activation)):
                emit_body(a2a_local=True)
            dsb = persist.tile([1, 16], F32)
            nc.vector.memset(dsb[:], 0.0)
            nc.sync.dma_start(dummy[:], dsb[:])
        else:
            for _ in range(repeat):
                emit_body(a2a_local=a2a_local)

    nc.compile()
    return nc


def _build_a2a_bench(k: int):
    """k back-to-back AllToAlls on the kernel's exchange buffer size."""
    nc = bacc.Bacc("TRN2", target_bir_lowering=False, debug=False,
                   num_devices=NCORES)
    src = nc.dram_tensor("src", [NCORES, HD, RPC], BF16,
                         kind="ExternalInput").ap()
    dst = nc.dram_tensor("dst", [1, 16], F32, kind="ExternalOutput").ap()
    with tile.TileContext(nc) as tc, ExitStack() as octx:
        dram = octx.enter_context(tc.tile_pool(name="dram", bufs=1,
                                               space="DRAM"))
        pool = octx.enter_context(tc.tile_pool(name="sb", bufs=1))
        a = dram.tile([NCORES, HD, RPC], BF16)
        bb = dram.tile([NCORES, HD, RPC], BF16)
        nc.sync.dma_start(a[:], src[:])
        bufs = [a, bb]
        for i in range(k):
            nc.gpsimd.collective_compute(
                "AllToAll", ALU.bypass,
                replica_groups=[list(range(NCORES))],
                ins=[bufs[i % 2][:]], outs=[bufs[(i + 1) % 2][:]],
            )
        dsb = pool.tile([1, 16], F32)
        nc.vector.memset(dsb[:], 0.0)
        nc.sync.dma_start(dst[:], dsb[:])
    nc.compile()
    return nc


def _make_runner(nc):
    """Jitted 8-core SPMD executor for a compiled Bass module."""
    install_neuronx_cc_hook()
    partition_name = nc.partition_id_tensor.name if nc.partition_id_tensor else None
    in_names, out_names, out_avals = [], [], []
    for alloc in nc.m.functions[0].allocations:
        if not isinstance(alloc, mybir.MemoryLocationSet):
            continue
        name = alloc.memorylocations[0].name
        if alloc.kind == "ExternalInput":
            if name != partition_name:
                in_names.append(name)
        elif alloc.kind == "ExternalOutput":
            out_names.append(name)
            out_avals.append(jax.core.ShapedArray(
                tuple(alloc.tensor_shape), mybir.dt.np(alloc.dtype)))
    n_params = len(in_names)
    n_outs = len(out_avals)
    all_in_names = list(in_names) + list(out_names)
    if partition_name is not None:
        all_in_names.append(partition_name)
    donate = tuple(range(n_params, n_params + n_outs))

    def _body(*args):
        operands = list(args)
        if partition_name is not None:
            operands.append(partition_id_tensor())
        return tuple(_bass_exec_p.bind(
            *operands,
            out_avals=tuple(out_avals),
            in_names=tuple(all_in_names),
            out_names=tuple(out_names),
            lowering_input_output_aliases=(),
            sim_require_finite=True,
            sim_require_nnan=True,
            nc=nc,
        ))

    devices = jax.devices()[:NCORES]
    mesh = Mesh(np.asarray(devices), ("core",))
    sharded = jax.jit(
        shard_map(_body, mesh=mesh,
                  in_specs=(PartitionSpec("core"),) * (n_params + n_outs),
                  out_specs=(PartitionSpec("core"),) * n_outs,
                  check_rep=False),
        donate_argnums=donate, keep_unused=True)

    zero_shapes = [a.shape for a in out_avals]
    zero_dtypes = [a.dtype for a in out_avals]

    def _zeros():
        return [np.zeros((NCORES * s[0], *s[1:]), d)
                for s, d in zip(zero_shapes, zero_dtypes)]

    def prepare(in_maps):
        """Concatenate per-core inputs and stage them on device once."""
        return [
            jax.device_put(np.concatenate(
                [np.asarray(m[name]) for m in in_maps], axis=0))
            for name in in_names
        ]

    def run_prepared(handles, as_numpy=True):
        out_arrs = sharded(*handles, *_zeros())
        if not as_numpy:
            jax.block_until_ready(out_arrs)
            return out_arrs
        return [
            {name: np.asarray(out_arrs[i]).reshape(NCORES, *zero_shapes[i])[c]
             for i, name in enumerate(out_names)}
            for c in range(NCORES)
        ]

    def run(in_maps):
        return run_prepared(prepare(in_maps))

    run.prepare = prepare
    run.run_prepared = run_prepared
    return run


def _shard_inputs(x, Wi, bi, Wo, bo, causal):
    """Host-side slicing/layout prep -> per-core input maps."""
    bf = ml_dtypes.bfloat16
    x = np.asarray(x, np.float32)
    Wi = np.asarray(Wi, np.float32)
    bi = np.asarray(bi, np.float32)
    Wo = np.asarray(Wo, np.float32)
    bo = np.asarray(bo, np.float32)

    xt = np.ascontiguousarray(x.reshape(R, D).T).astype(bf)       # (D, R)
    wot = np.ascontiguousarray(Wo.T).astype(bf)                   # (D, D)

    if causal:
        # additive mask pattern, accumulated onto diagonal score sub-blocks
        # via matmul(identity, m): scores[k, q] += m[k, q]; masked (k > q)
        # positions get -1e9 so exp underflows to exactly 0
        i = np.arange(KB)[:, None]
        j = np.arange(KB)[None, :]
        m = np.where(j >= i, 0.0, -1e9).astype(bf)
    else:
        m = np.zeros((KB, KB), bf)

    in_maps = []
    for c in range(NCORES):
        rows = np.concatenate([
            np.arange(c * HD, (c + 1) * HD),
            D + np.arange(c * HD, (c + 1) * HD),
            2 * D + np.arange(c * HD, (c + 1) * HD),
        ])
        wit_c = np.ascontiguousarray(Wi[rows].T).astype(bf)       # (D, 384)
        bi_c = np.ascontiguousarray(bi[rows]).astype(np.float32)  # (384,)
        in_maps.append({
            "xt": xt, "wit": wit_c, "bi_s": bi_c,
            "wot": wot, "bo_f": bo, "masks": m,
        })
    return in_maps


_CACHE = {}


def _get_runner(causal, repeat=1):
    key = (causal, repeat)
    if key not in _CACHE:
        nc = _build(causal, repeat)
        _CACHE[key] = _make_runner(nc)
    return _CACHE[key]


def kernel(x, Wi, bi, Wo, bo, causal_mask):
    causal = bool(int(np.asarray(causal_mask)))
    run = _get_runner(causal)
    in_maps = _shard_inputs(x, Wi, bi, Wo, bo, causal)
    res = run(in_maps)
    # res[c]["out_t"]: (D, RPC) fp32 = transposed rows [c*RPC, (c+1)*RPC)
    full = np.concatenate([res[c]["out_t"].T for c in range(NCORES)], axis=0)
    return np.ascontiguousarray(full.reshape(B, S, D).astype(np.float32))



# revision 54
# speedup vs baseline: 1.0845x; 1.0522x over previous
"""Multi-head causal attention (B=2, S=2048, D=1024, H=16) on 8 Trainium2 cores.

Sharding: tensor-parallel over heads. Core c computes QKV projection, causal
attention and softmax for heads {2c, 2c+1} over both batches, then an AllToAll
redistributes the attention output so core c owns rows [512c, 512c+512) of the
flattened (B*S, D) activation; each core applies the full output projection to
its row slice. Host code only slices/transposes inputs and concatenates the
per-core output slices.

All matmuls run in bf16 with fp32 PSUM accumulation. The pipeline works in
transposed layout ([dim, seq]) so that softmax reduces over the PSUM partition
axis via a ones-column folded into the PV matmul, and the attention output
lands directly in the layout the output projection consumes.

Key scheduling decisions (measured on HW via same-process A/B):
- Causal masking is additive, done on the PE (identity x mask-pattern matmul
  accumulated onto diagonal score blocks before exp), so the DVE is not on
  the scores->exp->PV critical path and exp underflows masked entries to 0.
- PV matmuls column-slice past fully-masked query prefixes (no memsets).
- The per-strip softmax normalization evacuates the PV psum accumulators to
  SBUF with two DVE copies first, freeing the banks for the next strip; the
  recip/gpsimd-broadcast/mul chain then runs off the critical path.
- Batch 1's QKV chains are deferred into closures fed into batch 0's
  (ACT-bound) attention stream at strip tails, filling PE idle windows.
- Output projection runs ob-outer/j-inner in 4 two-chain waves through the
  rotating score psum slots.
"""
import numpy as np
from contextlib import ExitStack

import jax
import ml_dtypes

import concourse.bass as bass
import concourse.tile as tile
from concourse import bacc, mybir
from concourse.bass2jax import (
    _bass_exec_p,
    install_neuronx_cc_hook,
    partition_id_tensor,
)
from jax.sharding import Mesh, PartitionSpec
from jax.experimental.shard_map import shard_map

B, S, D, H = 2, 2048, 1024, 16
DH = D // H            # 64
NCORES = 8
HPC = H // NCORES      # heads per core = 2
HD = HPC * DH          # head dims per core = 128
R = B * S              # flattened rows = 4096
RPC = R // NCORES      # rows per core after AllToAll = 512
QB = 512               # query block (also the AllToAll shard size)
KB = 128               # key block
NQB = S // QB          # 4 query blocks per batch
NKB = S // KB          # 16 key blocks per batch
CCH = D // 128         # contraction chunks for D-wide matmuls = 8

BF16 = mybir.dt.bfloat16
F32 = mybir.dt.float32
AF = mybir.ActivationFunctionType
ALU = mybir.AluOpType


def _build(causal: bool, repeat: int = 1, loop_n: int = 0,
           a2a_local: bool = False, parts: str = "full"):
    """Emit the SPMD Bass program (identical on all 8 cores).

    loop_n > 0 builds a timing variant: the whole per-iteration body runs
    inside a hardware For_i loop and the AllToAll is replaced by a local DMA
    copy (collectives cannot sit inside control flow), with the real output
    replaced by a tiny dummy (so the timing loop's donated output buffers are
    negligible to transfer). Used only to measure per-iteration device time.
    """
    timing = loop_n > 0
    nc = bacc.Bacc("TRN2", target_bir_lowering=False, debug=False,
                   num_devices=NCORES)

    xt = nc.dram_tensor("xt", [D, R], BF16, kind="ExternalInput").ap()
    wit = nc.dram_tensor("wit", [D, 3 * HD], BF16, kind="ExternalInput").ap()
    bi_s = nc.dram_tensor("bi_s", [3 * HD], F32, kind="ExternalInput").ap()
    wot = nc.dram_tensor("wot", [D, D], BF16, kind="ExternalInput").ap()
    bo_f = nc.dram_tensor("bo_f", [D], F32, kind="ExternalInput").ap()
    masks = nc.dram_tensor("masks", [KB, KB], BF16, kind="ExternalInput").ap()
    if timing:
        out_t = nc.dram_tensor("out_scratch", [D, RPC], F32).ap()
        dummy = nc.dram_tensor("tiny_out", [1, 16], F32, kind="ExternalOutput").ap()
    else:
        out_t = nc.dram_tensor("out_t", [D, RPC], F32, kind="ExternalOutput").ap()

    with tile.TileContext(nc) as tc, ExitStack() as octx:
        persist = octx.enter_context(tc.tile_pool(name="persist", bufs=1))
        dram = octx.enter_context(tc.tile_pool(name="dram", bufs=1, space="DRAM"))

        # ---- persistent SBUF state (x chunks queued right after wit: the
        # QKV matmuls need them first; wot/bo only matter at the end) ----
        wit_sb = persist.tile([128, CCH, 3 * HD], BF16)
        nc.sync.dma_start(wit_sb[:], wit.rearrange("(cc p) n -> p cc n", p=128))
        bias_sb = persist.tile([128, 3], F32)
        nc.sync.dma_start(bias_sb[:], bi_s.rearrange("(t p) -> p t", p=128))
        xt_pool = octx.enter_context(tc.tile_pool(name="xt_pool", bufs=1))
        xt_sb = xt_pool.tile([128, CCH, R], BF16)
        xt_r = xt.rearrange("(cc p) r -> p cc r", p=128)
        for cc in range(CCH):
            nc.sync.dma_start(xt_sb[:, cc, :], xt_r[:, cc, :])
        wot_sb = persist.tile([128, CCH, D], BF16)
        nc.sync.dma_start(wot_sb[:], wot.rearrange("(cc p) o -> p cc o", p=128))
        bo_sb = persist.tile([128, CCH], F32)
        nc.sync.dma_start(bo_sb[:], bo_f.rearrange("(oc p) -> p oc", p=128))
        # one [128,128] triangle: mask[i, j] = (j >= i), same for every
        # diagonal sub-block once the exp is column-sliced
        mask_sb = persist.tile([128, KB], BF16)
        if causal:
            nc.sync.dma_start(mask_sb[:], masks[:])

        identity = persist.tile([128, 128], BF16)
        from concourse.masks import make_identity
        make_identity(nc, identity[:])
        ones_row = persist.tile([1, 128], BF16)
        nc.vector.memset(ones_row[:], 1.0)

        # qT/kT: [head-dims (2 heads x 64), S] per batch; v: [k rows, 65] blocks
        qt_sb = [persist.tile([128, S], BF16, name=f"qt{b}") for b in range(B)]
        kt_sb = [persist.tile([128, S], BF16, name=f"kt{b}") for b in range(B)]
        # v_sb[h][:, g, 0:64] = v rows for global k-block g; col 64 = 1.0
        v_sb = [persist.tile([128, B * NKB, DH + 1], BF16, name=f"v{h}")
                for h in range(HPC)]
        for h in range(HPC):
            nc.vector.memset(v_sb[h][:, :, DH:DH + 1], 1.0)

        a2a_in = dram.tile([NCORES, HD, RPC], BF16)
        a2a_out = dram.tile([NCORES, HD, RPC], BF16)
        ao_sb = persist.tile([128, NCORES, RPC], BF16, name="ao_sb")

        # one PSUM pool for all phases; tags share slots so cross-phase reuse
        # only waits on the previous user of one slot, not a whole phase.
        # banks: big0(2) + big1(2) + tr(2) + o0(1) + o1(1) = 8
        psum = octx.enter_context(tc.tile_pool(name="psum", bufs=1,
                                               space="PSUM"))
        work = octx.enter_context(tc.tile_pool(name="work", bufs=3))
        epool = octx.enter_context(tc.tile_pool(name="epool", bufs=6))

        def big_ps(i, name):
            # [128, 1024] = 2 PSUM banks; halves are used as two 512-wide
            # chains / score blocks so ACT+DVE consumers see one wide AP.
            # One tag, 3 rotating slots (6 banks): the head-interleaved score
            # stream gets effective double-buffering without per-head tags.
            return psum.tile([128, 2 * QB], F32, tag="big", bufs=3,
                             name=name)

        def emit_body(a2a_local: bool):
            # ================= QKV projection (transposed) =================
            vt_tiles = {}

            def qk_chain_unit(b, tsr, pp):
                """One [128,1024] psum tile = two 512-row chains (rows
                2pp*QB..), contraction inner. Drained by one DVE bias add."""
                def run():
                    if tsr == 2 and b not in vt_tiles:
                        vt_tiles[b] = work.tile([128, S], BF16, tag="vt",
                                                name=f"vt{b}")
                    ps = big_ps(0, "ps_qkv")
                    for cc in range(CCH):
                        for rc in (2 * pp, 2 * pp + 1):
                            r0 = b * S + rc * QB
                            half = (rc % 2) * QB
                            nc.tensor.matmul(
                                ps[:, half:half + QB],
                                wit_sb[:, cc, tsr * HD:(tsr + 1) * HD],
                                xt_sb[:, cc, r0:r0 + QB],
                                start=(cc == 0), stop=(cc == CCH - 1),
                            )
                    dst = (qt_sb[b] if tsr == 0 else
                           kt_sb[b] if tsr == 1 else vt_tiles[b])
                    nc.vector.tensor_scalar(
                        dst[:, pp * 2 * QB:(pp + 1) * 2 * QB], ps[:],
                        bias_sb[:, tsr:tsr + 1], None, ALU.add)
                return run

            def vtr_unit(b, rb0, psum_tag):
                """Transpose vT blocks rb0..rb0+3 -> v_sb [k rows, dims]."""
                def run():
                    for rb in range(rb0, rb0 + 4):
                        if psum_tag == "big":
                            pst = psum.tile([128, 128], BF16, tag="big",
                                            bufs=3, name="ps_tr")
                        else:
                            pst = psum.tile([128, 128], BF16,
                                            tag=f"o{rb % 2}",
                                            bufs=1, name="ps_tr")
                        nc.tensor.transpose(
                            pst[:],
                            vt_tiles[b][:, rb * 128:(rb + 1) * 128],
                            identity[:])
                        g = b * NKB + rb
                        for h in range(HPC):
                            nc.vector.tensor_copy(
                                v_sb[h][:, g, 0:DH],
                                pst[:, h * DH:(h + 1) * DH])
                return run

            def qkv_units(b, psum_tag="o"):
                us = []
                for tsr in range(3):
                    for pp in range(2):
                        us.append(qk_chain_unit(b, tsr, pp))
                for rb0 in range(0, S // 128, 4):
                    us.append(vtr_unit(b, rb0, psum_tag))
                return us

            for u in qkv_units(0):
                u()
            if parts == "qkv":
                for u in qkv_units(1):
                    u()
                return
            # batch 1's projection is deferred: its units are fed one at a
            # time into batch 0's (ACT-bound) attention stream so the PE
            # fills its exp-wait and normalize-tail gaps with useful work
            pending = qkv_units(1, psum_tag="big")

            def feed(n=1):
                for _ in range(n):
                    if pending:
                        pending.pop(0)()

            # ======================= attention =============================
            # scores run one k-block ahead of PV so PE never waits on exp
            for b in range(B):
                if b == 1:
                    while pending:
                        feed()
                for qb in range(NQB):
                    nkb = 4 * (qb + 1) if causal else NKB
                    q0 = qb * QB
                    ps_o = [psum.tile([DH + 1, QB], F32, tag=f"o{h}", bufs=1,
                                      name=f"ps_o{h}")
                            for h in range(HPC)]

                    def scores_pair(p):
                        """Two k-blocks (2p, 2p+1) -> one [128,1024] psum per
                        head, a single exp. Causal masking is folded into the
                        scores additively: for diagonal sub-blocks one extra
                        matmul (identity stationary x additive-mask pattern)
                        accumulates -1e9 onto masked positions, so exp yields
                        exact zeros with no DVE involvement. MMs alternate
                        heads so consecutive matmuls land on different PE
                        row-groups and overlap in the array."""
                        pss = [big_ps(h, f"ps_s{h}") for h in range(HPC)]
                        for half in range(2):
                            kb = 2 * p + half
                            rel = kb - 4 * qb
                            diag = (causal and "nodiag" not in parts
                                    and 0 <= rel <= 3)
                            for h in range(HPC):
                                nc.tensor.matmul(
                                    pss[h][:, half * QB:(half + 1) * QB],
                                    kt_sb[b][h * DH:(h + 1) * DH,
                                             kb * KB:(kb + 1) * KB],
                                    qt_sb[b][h * DH:(h + 1) * DH, q0:q0 + QB],
                                    start=True, stop=not diag,
                                )
                            if diag:
                                c0 = half * QB + rel * KB
                                for h in range(HPC):
                                    nc.tensor.matmul(
                                        pss[h][:, c0:c0 + KB],
                                        identity[:], mask_sb[:],
                                        start=False, stop=True,
                                        skip_group_check=True,
                                    )
                        es = []
                        for h in range(HPC):
                            ps_s = pss[h]
                            e = epool.tile([128, 2 * QB], BF16, tag="expT",
                                           name="expT")
                            if "peonly" in parts:
                                es.append(None)
                                continue
                            if "noexp" in parts:
                                # tiny write so the tile is allocated; PV
                                # reads garbage (timing probe only)
                                nc.gpsimd.memset(e[:, 0:1], 0.0)
                                es.append(e)
                                continue
                            t = 2 * p - 4 * qb if causal else -1
                            if "nodiag" in parts:
                                t = -1
                            if causal and t >= 0:
                                # halves are diagonal blocks t and t+1.
                                # Fully-masked prefixes of each half are
                                # never exp'd or zeroed: the PV matmuls
                                # column-slice past them instead. Triangle
                                # masking already happened additively in the
                                # scores psum.
                                c0 = t * KB
                                nc.scalar.activation(
                                    e[:, c0:2 * QB], ps_s[:, c0:2 * QB],
                                    AF.Exp, scale=1.0 / 8.0)
                            else:
                                nc.scalar.activation(e[:], ps_s[:], AF.Exp,
                                                     scale=1.0 / 8.0)
                            es.append(e)
                        return es

                    def pv_pair(p, es):
                        for h in range(HPC):
                            for half in range(2):
                                kb = 2 * p + half
                                # causal: queries before co see only masked
                                # keys in this block -> skip those columns
                                co = (max(0, (2 * p + half - 4 * qb) * KB)
                                      if (causal and "nodiag" not in parts)
                                      else 0)
                                if es[h] is None:
                                    rhs = qt_sb[b][:, q0:q0 + QB]
                                    co = 0
                                else:
                                    rhs = es[h][:, half * QB + co:
                                                (half + 1) * QB]
                                nc.tensor.matmul(
                                    ps_o[h][:, co:QB],
                                    v_sb[h][:, b * NKB + kb, :],
                                    rhs,
                                    start=(kb == 0), stop=(kb == nkb - 1),
                                )

                    npair = nkb // 2
                    es_prev = scores_pair(0)
                    for p in range(1, npair):
                        es = scores_pair(p)
                        pv_pair(p - 1, es_prev)
                        es_prev = es
                    pv_pair(npair - 1, es_prev)

                    at = work.tile([128, QB], BF16, tag="attnT", name="attnT")
                    if "nonorm" in parts:
                        nc.vector.tensor_copy(at[0:DH, :], ps_o[0][0:DH, :])
                        nc.vector.tensor_copy(at[DH:2 * DH, :],
                                              ps_o[1][0:DH, :])
                    else:
                        # evacuate the PV accumulators (values + denominator
                        # row) to SBUF immediately so the psum banks free up
                        # for the next strip's PV; then the broadcast and the
                        # normalize multiplies run on the otherwise-idle
                        # GPSIMD engine so the DVE FIFO stays clear for the
                        # next strip's triangle-mask muls (which gate PV).
                        last = (b == B - 1 and qb == NQB - 1)
                        rcr = work.tile([1, 2 * QB], F32, tag="rcr",
                                        name="rcr")
                        if last:
                            # nothing competes for these psum banks after the
                            # final strip: skip the evacuation copies and
                            # normalize straight out of psum (shorter chain
                            # before the exchange + output projection)
                            for h in range(HPC):
                                nc.vector.reciprocal(
                                    rcr[0:1, h * QB:(h + 1) * QB],
                                    ps_o[h][DH:DH + 1, :])
                            vals = [ps_o[h][0:DH, :] for h in range(HPC)]
                        else:
                            pvs = work.tile([128, QB], F32, tag="pvs",
                                            name="pvs")
                            rc = work.tile([1, 2 * QB], F32, tag="rc",
                                           name="rc")
                            for h in range(HPC):
                                nc.vector.tensor_copy(
                                    pvs[h * DH:(h + 1) * DH, :],
                                    ps_o[h][0:DH, :])
                                nc.vector.tensor_copy(
                                    rc[0:1, h * QB:(h + 1) * QB],
                                    ps_o[h][DH:DH + 1, :])
                            nc.vector.reciprocal(rcr[:], rc[:])
                            vals = [pvs[h * DH:(h + 1) * DH, :]
                                    for h in range(HPC)]
                        rpb = work.tile([128, 2 * QB], F32, tag="rpb",
                                        name="rpb")
                        nc.gpsimd.partition_broadcast(rpb[:], rcr[0:1, :])
                        for h in range(HPC):
                            nc.vector.tensor_mul(
                                at[h * DH:(h + 1) * DH, :],
                                vals[h],
                                rpb[h * DH:(h + 1) * DH,
                                    h * QB:(h + 1) * QB])
                    j = b * NQB + qb
                    nc.sync.dma_start(a2a_in[j], at[:])
                    if a2a_local:
                        # timing stand-in for the exchange (collectives can't
                        # sit in control flow): the real kernel's data path is
                        # at -> a2a_in -> [AllToAll, charged separately by the
                        # harness] -> ao_sb, so the local build mirrors just
                        # the two endpoint DMAs
                        nc.sync.dma_start(ao_sb[:, j, :], a2a_in[j])
                    if b == 0:
                        # fill the strip-tail normalize window with deferred
                        # batch-1 projection work (exactly one unit: a second
                        # one overruns the idle window and stalls the next
                        # strip's exp stream)
                        feed()

            # ================= AllToAll + output projection ================
            if parts == "qkv+att":
                return
            if not a2a_local:
                nc.gpsimd.collective_compute(
                    "AllToAll", ALU.bypass,
                    replica_groups=[list(range(NCORES))],
                    ins=[a2a_in[:]], outs=[a2a_out[:]],
                )
                for j in range(NCORES):
                    nc.sync.dma_start(ao_sb[:, j, :], a2a_out[j])
            # 4 waves of 2 output chains each through the 3 rotating big
            # psum slots. The first 3 waves run their j=0..6 accumulation
            # matmuls up front (those shards arrived before the last strip)
            # and defer the j=7 matmuls to the end, so the PE keeps working
            # under the last strip's normalize/exchange tail instead of
            # head-of-line blocking on ao_sb[7].
            def opj_mm(ps, w, j, start, stop):
                for half in range(2):
                    ob = 2 * w + half
                    nc.tensor.matmul(
                        ps[:, half * RPC:(half + 1) * RPC],
                        wot_sb[:, j, ob * 128:(ob + 1) * 128],
                        ao_sb[:, j, :],
                        start=start, stop=stop,
                    )

            def opj_drain(ps, w):
                os = work.tile([128, 2 * RPC], F32, tag="os", name="os")
                for half in range(2):
                    ob = 2 * w + half
                    nc.vector.tensor_scalar(
                        os[:, half * RPC:(half + 1) * RPC],
                        ps[:, half * RPC:(half + 1) * RPC],
                        bo_sb[:, ob:ob + 1], None, ALU.add)
                    nc.sync.dma_start(out_t[ob * 128:(ob + 1) * 128, :],
                                      os[:, half * RPC:(half + 1) * RPC])

            waves = []
            for w in range(3):
                ps = big_ps(w, f"ps_outp{w}")
                for j in range(NCORES - 1):
                    opj_mm(ps, w, j, start=(j == 0), stop=False)
                waves.append(ps)
            for w in range(3):
                opj_mm(waves[w], w, NCORES - 1, start=False, stop=True)
                opj_drain(waves[w], w)
            ps = big_ps(3, "ps_outp3")
            for j in range(NCORES):
                opj_mm(ps, 3, j, start=(j == 0), stop=(j == NCORES - 1))
            opj_drain(ps, 3)

        if loop_n:
            with tc.For_i(0, loop_n, 1,
                          hint_engines=(mybir.EngineType.PE,
                                        mybir.EngineType.DVE,
                                        mybir.EngineType.Activation)):
                emit_body(a2a_local=True)
            dsb = persist.tile([1, 16], F32)
            nc.vector.memset(dsb[:], 0.0)
            nc.sync.dma_start(dummy[:], dsb[:])
        else:
            for _ in range(repeat):
                emit_body(a2a_local=a2a_local)

    nc.compile()
    return nc


def _build_a2a_bench(k: int):
    """k back-to-back AllToAlls on the kernel's exchange buffer size."""
    nc = bacc.Bacc("TRN2", target_bir_lowering=False, debug=False,
                   num_devices=NCORES)
    src = nc.dram_tensor("src", [NCORES, HD, RPC], BF16,
                         kind="ExternalInput").ap()
    dst = nc.dram_tensor("dst", [1, 16], F32, kind="ExternalOutput").ap()
    with tile.TileContext(nc) as tc, ExitStack() as octx:
        dram = octx.enter_context(tc.tile_pool(name="dram", bufs=1,
                                               space="DRAM"))
        pool = octx.enter_context(tc.tile_pool(name="sb", bufs=1))
        a = dram.tile([NCORES, HD, RPC], BF16)
        bb = dram.tile([NCORES, HD, RPC], BF16)
        nc.sync.dma_start(a[:], src[:])
        bufs = [a, bb]
        for i in range(k):
            nc.gpsimd.collective_compute(
                "AllToAll", ALU.bypass,
                replica_groups=[list(range(NCORES))],
                ins=[bufs[i % 2][:]], outs=[bufs[(i + 1) % 2][:]],
            )
        dsb = pool.tile([1, 16], F32)
        nc.vector.memset(dsb[:], 0.0)
        nc.sync.dma_start(dst[:], dsb[:])
    nc.compile()
    return nc


def _make_runner(nc):
    """Jitted 8-core SPMD executor for a compiled Bass module."""
    install_neuronx_cc_hook()
    partition_name = nc.partition_id_tensor.name if nc.partition_id_tensor else None
    in_names, out_names, out_avals = [], [], []
    for alloc in nc.m.functions[0].allocations:
        if not isinstance(alloc, mybir.MemoryLocationSet):
            continue
        name = alloc.memorylocations[0].name
        if alloc.kind == "ExternalInput":
            if name != partition_name:
                in_names.append(name)
        elif alloc.kind == "ExternalOutput":
            out_names.append(name)
            out_avals.append(jax.core.ShapedArray(
                tuple(alloc.tensor_shape), mybir.dt.np(alloc.dtype)))
    n_params = len(in_names)
    n_outs = len(out_avals)
    all_in_names = list(in_names) + list(out_names)
    if partition_name is not None:
        all_in_names.append(partition_name)
    donate = tuple(range(n_params, n_params + n_outs))

    def _body(*args):
        operands = list(args)
        if partition_name is not None:
            operands.append(partition_id_tensor())
        return tuple(_bass_exec_p.bind(
            *operands,
            out_avals=tuple(out_avals),
            in_names=tuple(all_in_names),
            out_names=tuple(out_names),
            lowering_input_output_aliases=(),
            sim_require_finite=True,
            sim_require_nnan=True,
            nc=nc,
        ))

    devices = jax.devices()[:NCORES]
    mesh = Mesh(np.asarray(devices), ("core",))
    sharded = jax.jit(
        shard_map(_body, mesh=mesh,
                  in_specs=(PartitionSpec("core"),) * (n_params + n_outs),
                  out_specs=(PartitionSpec("core"),) * n_outs,
                  check_rep=False),
        donate_argnums=donate, keep_unused=True)

    zero_shapes = [a.shape for a in out_avals]
    zero_dtypes = [a.dtype for a in out_avals]

    def _zeros():
        return [np.zeros((NCORES * s[0], *s[1:]), d)
                for s, d in zip(zero_shapes, zero_dtypes)]

    def prepare(in_maps):
        """Concatenate per-core inputs and stage them on device once."""
        return [
            jax.device_put(np.concatenate(
                [np.asarray(m[name]) for m in in_maps], axis=0))
            for name in in_names
        ]

    def run_prepared(handles, as_numpy=True):
        out_arrs = sharded(*handles, *_zeros())
        if not as_numpy:
            jax.block_until_ready(out_arrs)
            return out_arrs
        return [
            {name: np.asarray(out_arrs[i]).reshape(NCORES, *zero_shapes[i])[c]
             for i, name in enumerate(out_names)}
            for c in range(NCORES)
        ]

    def run(in_maps):
        return run_prepared(prepare(in_maps))

    run.prepare = prepare
    run.run_prepared = run_prepared
    return run


def _shard_inputs(x, Wi, bi, Wo, bo, causal):
    """Host-side slicing/layout prep -> per-core input maps."""
    bf = ml_dtypes.bfloat16
    x = np.asarray(x, np.float32)
    Wi = np.asarray(Wi, np.float32)
    bi = np.asarray(bi, np.float32)
    Wo = np.asarray(Wo, np.float32)
    bo = np.asarray(bo, np.float32)

    xt = np.ascontiguousarray(x.reshape(R, D).T).astype(bf)       # (D, R)
    wot = np.ascontiguousarray(Wo.T).astype(bf)                   # (D, D)

    if causal:
        # additive mask pattern, accumulated onto diagonal score sub-blocks
        # via matmul(identity, m): scores[k, q] += m[k, q]; masked (k > q)
        # positions get -1e9 so exp underflows to exactly 0
        i = np.arange(KB)[:, None]
        j = np.arange(KB)[None, :]
        m = np.where(j >= i, 0.0, -1e9).astype(bf)
    else:
        m = np.zeros((KB, KB), bf)

    in_maps = []
    for c in range(NCORES):
        rows = np.concatenate([
            np.arange(c * HD, (c + 1) * HD),
            D + np.arange(c * HD, (c + 1) * HD),
            2 * D + np.arange(c * HD, (c + 1) * HD),
        ])
        wit_c = np.ascontiguousarray(Wi[rows].T).astype(bf)       # (D, 384)
        bi_c = np.ascontiguousarray(bi[rows]).astype(np.float32)  # (384,)
        in_maps.append({
            "xt": xt, "wit": wit_c, "bi_s": bi_c,
            "wot": wot, "bo_f": bo, "masks": m,
        })
    return in_maps


_CACHE = {}


def _get_runner(causal, repeat=1):
    key = (causal, repeat)
    if key not in _CACHE:
        nc = _build(causal, repeat)
        _CACHE[key] = _make_runner(nc)
    return _CACHE[key]


def kernel(x, Wi, bi, Wo, bo, causal_mask):
    causal = bool(int(np.asarray(causal_mask)))
    run = _get_runner(causal)
    in_maps = _shard_inputs(x, Wi, bi, Wo, bo, causal)
    res = run(in_maps)
    # res[c]["out_t"]: (D, RPC) fp32 = transposed rows [c*RPC, (c+1)*RPC)
    full = np.concatenate([res[c]["out_t"].T for c in range(NCORES)], axis=0)
    return np.ascontiguousarray(full.reshape(B, S, D).astype(np.float32))



# revision 56
# speedup vs baseline: 1.0906x; 1.0056x over previous
"""Multi-head causal attention (B=2, S=2048, D=1024, H=16) on 8 Trainium2 cores.

Sharding: tensor-parallel over heads. Core c computes QKV projection, causal
attention and softmax for heads {2c, 2c+1} over both batches, then an AllToAll
redistributes the attention output so core c owns rows [512c, 512c+512) of the
flattened (B*S, D) activation; each core applies the full output projection to
its row slice. Host code only slices/transposes inputs and concatenates the
per-core output slices.

All matmuls run in bf16 with fp32 PSUM accumulation. The pipeline works in
transposed layout ([dim, seq]) so that softmax reduces over the PSUM partition
axis via a ones-column folded into the PV matmul, and the attention output
lands directly in the layout the output projection consumes.

Key scheduling decisions (measured on HW via same-process A/B):
- Causal masking is additive, done on the PE (identity x mask-pattern matmul
  accumulated onto diagonal score blocks before exp), so the DVE is not on
  the scores->exp->PV critical path and exp underflows masked entries to 0.
- PV matmuls column-slice past fully-masked query prefixes (no memsets).
- The per-strip softmax normalization evacuates the PV psum accumulators to
  SBUF with two DVE copies first, freeing the banks for the next strip; the
  recip/gpsimd-broadcast/mul chain then runs off the critical path.
- Batch 1's QKV chains are deferred into closures fed into batch 0's
  (ACT-bound) attention stream at strip tails, filling PE idle windows.
- Output projection runs ob-outer/j-inner in 4 two-chain waves through the
  rotating score psum slots.
"""
import numpy as np
from contextlib import ExitStack

import jax
import ml_dtypes

import concourse.bass as bass
import concourse.tile as tile
from concourse import bacc, mybir
from concourse.bass2jax import (
    _bass_exec_p,
    install_neuronx_cc_hook,
    partition_id_tensor,
)
from jax.sharding import Mesh, PartitionSpec
from jax.experimental.shard_map import shard_map

B, S, D, H = 2, 2048, 1024, 16
DH = D // H            # 64
NCORES = 8
HPC = H // NCORES      # heads per core = 2
HD = HPC * DH          # head dims per core = 128
R = B * S              # flattened rows = 4096
RPC = R // NCORES      # rows per core after AllToAll = 512
QB = 512               # query block (also the AllToAll shard size)
KB = 128               # key block
NQB = S // QB          # 4 query blocks per batch
NKB = S // KB          # 16 key blocks per batch
CCH = D // 128         # contraction chunks for D-wide matmuls = 8

BF16 = mybir.dt.bfloat16
F32 = mybir.dt.float32
AF = mybir.ActivationFunctionType
ALU = mybir.AluOpType


def _build(causal: bool, repeat: int = 1, loop_n: int = 0,
           a2a_local: bool = False, parts: str = "full"):
    """Emit the SPMD Bass program (identical on all 8 cores).

    loop_n > 0 builds a timing variant: the whole per-iteration body runs
    inside a hardware For_i loop and the AllToAll is replaced by a local DMA
    copy (collectives cannot sit inside control flow), with the real output
    replaced by a tiny dummy (so the timing loop's donated output buffers are
    negligible to transfer). Used only to measure per-iteration device time.
    """
    timing = loop_n > 0
    nc = bacc.Bacc("TRN2", target_bir_lowering=False, debug=False,
                   num_devices=NCORES)

    xt = nc.dram_tensor("xt", [D, R], BF16, kind="ExternalInput").ap()
    wit = nc.dram_tensor("wit", [D, 3 * HD], BF16, kind="ExternalInput").ap()
    bi_s = nc.dram_tensor("bi_s", [3 * HD], F32, kind="ExternalInput").ap()
    wot = nc.dram_tensor("wot", [D, D], BF16, kind="ExternalInput").ap()
    bo_f = nc.dram_tensor("bo_f", [D], F32, kind="ExternalInput").ap()
    masks = nc.dram_tensor("masks", [KB, KB], BF16, kind="ExternalInput").ap()
    if timing:
        out_t = nc.dram_tensor("out_scratch", [D, RPC], F32).ap()
        dummy = nc.dram_tensor("tiny_out", [1, 16], F32, kind="ExternalOutput").ap()
    else:
        out_t = nc.dram_tensor("out_t", [D, RPC], F32, kind="ExternalOutput").ap()

    with tile.TileContext(nc) as tc, ExitStack() as octx:
        persist = octx.enter_context(tc.tile_pool(name="persist", bufs=1))
        dram = octx.enter_context(tc.tile_pool(name="dram", bufs=1, space="DRAM"))

        # ---- persistent SBUF state (x chunks queued right after wit: the
        # QKV matmuls need them first; wot/bo only matter at the end) ----
        wit_sb = persist.tile([128, CCH, 3 * HD], BF16)
        nc.sync.dma_start(wit_sb[:], wit.rearrange("(cc p) n -> p cc n", p=128))
        bias_sb = persist.tile([128, 3], F32)
        nc.sync.dma_start(bias_sb[:], bi_s.rearrange("(t p) -> p t", p=128))
        xt_pool = octx.enter_context(tc.tile_pool(name="xt_pool", bufs=1))
        xt_sb = xt_pool.tile([128, CCH, R], BF16)
        xt_r = xt.rearrange("(cc p) r -> p cc r", p=128)
        for cc in range(CCH):
            nc.sync.dma_start(xt_sb[:, cc, :], xt_r[:, cc, :])
        wot_sb = persist.tile([128, CCH, D], BF16)
        nc.sync.dma_start(wot_sb[:], wot.rearrange("(cc p) o -> p cc o", p=128))
        bo_sb = persist.tile([128, CCH], F32)
        nc.sync.dma_start(bo_sb[:], bo_f.rearrange("(oc p) -> p oc", p=128))
        # one [128,128] triangle: mask[i, j] = (j >= i), same for every
        # diagonal sub-block once the exp is column-sliced
        mask_sb = persist.tile([128, KB], BF16)
        if causal:
            nc.sync.dma_start(mask_sb[:], masks[:])

        identity = persist.tile([128, 128], BF16)
        from concourse.masks import make_identity
        make_identity(nc, identity[:])
        ones_row = persist.tile([1, 128], BF16)
        nc.vector.memset(ones_row[:], 1.0)

        # qT/kT: [head-dims (2 heads x 64), S] per batch; v: [k rows, 65] blocks
        qt_sb = [persist.tile([128, S], BF16, name=f"qt{b}") for b in range(B)]
        kt_sb = [persist.tile([128, S], BF16, name=f"kt{b}") for b in range(B)]
        # v_sb[h][:, g, 0:64] = v rows for global k-block g; col 64 = 1.0
        v_sb = [persist.tile([128, B * NKB, DH + 1], BF16, name=f"v{h}")
                for h in range(HPC)]
        for h in range(HPC):
            nc.vector.memset(v_sb[h][:, :, DH:DH + 1], 1.0)

        a2a_in = dram.tile([NCORES, HD, RPC], BF16)
        a2a_out = dram.tile([NCORES, HD, RPC], BF16)
        ao_sb = persist.tile([128, NCORES, RPC], BF16, name="ao_sb")

        # one PSUM pool for all phases; tags share slots so cross-phase reuse
        # only waits on the previous user of one slot, not a whole phase.
        # banks: big0(2) + big1(2) + tr(2) + o0(1) + o1(1) = 8
        psum = octx.enter_context(tc.tile_pool(name="psum", bufs=1,
                                               space="PSUM"))
        work = octx.enter_context(tc.tile_pool(name="work", bufs=3))
        epool = octx.enter_context(tc.tile_pool(name="epool", bufs=6))

        def big_ps(i, name):
            # [128, 1024] = 2 PSUM banks; halves are used as two 512-wide
            # chains / score blocks so ACT+DVE consumers see one wide AP.
            # One tag, 3 rotating slots (6 banks): the head-interleaved score
            # stream gets effective double-buffering without per-head tags.
            return psum.tile([128, 2 * QB], F32, tag="big", bufs=3,
                             name=name)

        def emit_body(a2a_local: bool):
            # ================= QKV projection (transposed) =================
            vt_tiles = {}

            def qk_chain_unit(b, tsr, pp):
                """One [128,1024] psum tile = two 512-row chains (rows
                2pp*QB..), contraction inner. Drained by one DVE bias add."""
                def run():
                    if tsr == 2 and b not in vt_tiles:
                        vt_tiles[b] = work.tile([128, S], BF16, tag="vt",
                                                name=f"vt{b}")
                    ps = big_ps(0, "ps_qkv")
                    for cc in range(CCH):
                        for rc in (2 * pp, 2 * pp + 1):
                            r0 = b * S + rc * QB
                            half = (rc % 2) * QB
                            nc.tensor.matmul(
                                ps[:, half:half + QB],
                                wit_sb[:, cc, tsr * HD:(tsr + 1) * HD],
                                xt_sb[:, cc, r0:r0 + QB],
                                start=(cc == 0), stop=(cc == CCH - 1),
                            )
                    dst = (qt_sb[b] if tsr == 0 else
                           kt_sb[b] if tsr == 1 else vt_tiles[b])
                    nc.vector.tensor_scalar(
                        dst[:, pp * 2 * QB:(pp + 1) * 2 * QB], ps[:],
                        bias_sb[:, tsr:tsr + 1], None, ALU.add)
                return run

            def vtr_unit(b, rb0, psum_tag):
                """Transpose vT blocks rb0..rb0+3 -> v_sb [k rows, dims]."""
                def run():
                    for rb in range(rb0, rb0 + 4):
                        if psum_tag == "big":
                            pst = psum.tile([128, 128], BF16, tag="big",
                                            bufs=3, name="ps_tr")
                        else:
                            pst = psum.tile([128, 128], BF16,
                                            tag=f"o{rb % 2}",
                                            bufs=1, name="ps_tr")
                        nc.tensor.transpose(
                            pst[:],
                            vt_tiles[b][:, rb * 128:(rb + 1) * 128],
                            identity[:])
                        g = b * NKB + rb
                        for h in range(HPC):
                            nc.vector.tensor_copy(
                                v_sb[h][:, g, 0:DH],
                                pst[:, h * DH:(h + 1) * DH])
                return run

            def qkv_units(b, psum_tag="o"):
                us = []
                for tsr in range(3):
                    for pp in range(2):
                        us.append(qk_chain_unit(b, tsr, pp))
                for rb0 in range(0, S // 128, 4):
                    us.append(vtr_unit(b, rb0, psum_tag))
                return us

            for u in qkv_units(0):
                u()
            if parts == "qkv":
                for u in qkv_units(1):
                    u()
                return
            # batch 1's projection is deferred: its units are fed one at a
            # time into batch 0's (ACT-bound) attention stream so the PE
            # fills its exp-wait and normalize-tail gaps with useful work
            pending = qkv_units(1, psum_tag="big")

            def feed(n=1):
                for _ in range(n):
                    if pending:
                        pending.pop(0)()

            # ======================= attention =============================
            # scores run one k-block ahead of PV so PE never waits on exp
            for b in range(B):
                if b == 1:
                    while pending:
                        feed()
                for qb in range(NQB):
                    nkb = 4 * (qb + 1) if causal else NKB
                    q0 = qb * QB
                    ps_o = [psum.tile([DH + 1, QB], F32, tag=f"o{h}", bufs=1,
                                      name=f"ps_o{h}")
                            for h in range(HPC)]

                    def scores_pair(p):
                        """Two k-blocks (2p, 2p+1) -> one [128,1024] psum per
                        head, a single exp. Causal masking is folded into the
                        scores additively: for diagonal sub-blocks one extra
                        matmul (identity stationary x additive-mask pattern)
                        accumulates -1e9 onto masked positions, so exp yields
                        exact zeros with no DVE involvement. MMs alternate
                        heads so consecutive matmuls land on different PE
                        row-groups and overlap in the array."""
                        pss = [big_ps(h, f"ps_s{h}") for h in range(HPC)]
                        for half in range(2):
                            kb = 2 * p + half
                            rel = kb - 4 * qb
                            diag = (causal and "nodiag" not in parts
                                    and 0 <= rel <= 3)
                            for h in range(HPC):
                                nc.tensor.matmul(
                                    pss[h][:, half * QB:(half + 1) * QB],
                                    kt_sb[b][h * DH:(h + 1) * DH,
                                             kb * KB:(kb + 1) * KB],
                                    qt_sb[b][h * DH:(h + 1) * DH, q0:q0 + QB],
                                    start=True, stop=not diag,
                                )
                            if diag:
                                c0 = half * QB + rel * KB
                                for h in range(HPC):
                                    nc.tensor.matmul(
                                        pss[h][:, c0:c0 + KB],
                                        identity[:], mask_sb[:],
                                        start=False, stop=True,
                                        skip_group_check=True,
                                    )
                        es = []
                        for h in range(HPC):
                            ps_s = pss[h]
                            e = epool.tile([128, 2 * QB], BF16, tag="expT",
                                           name="expT")
                            if "peonly" in parts:
                                es.append(None)
                                continue
                            if "noexp" in parts:
                                # tiny write so the tile is allocated; PV
                                # reads garbage (timing probe only)
                                nc.gpsimd.memset(e[:, 0:1], 0.0)
                                es.append(e)
                                continue
                            t = 2 * p - 4 * qb if causal else -1
                            if "nodiag" in parts:
                                t = -1
                            if causal and t >= 0:
                                # halves are diagonal blocks t and t+1.
                                # Fully-masked prefixes of each half are
                                # never exp'd or zeroed: the PV matmuls
                                # column-slice past them instead. Triangle
                                # masking already happened additively in the
                                # scores psum.
                                c0 = t * KB
                                nc.scalar.activation(
                                    e[:, c0:2 * QB], ps_s[:, c0:2 * QB],
                                    AF.Exp, scale=1.0 / 8.0)
                            else:
                                nc.scalar.activation(e[:], ps_s[:], AF.Exp,
                                                     scale=1.0 / 8.0)
                            es.append(e)
                        return es

                    def pv_pair(p, es):
                        for h in range(HPC):
                            for half in range(2):
                                kb = 2 * p + half
                                # causal: queries before co see only masked
                                # keys in this block -> skip those columns
                                co = (max(0, (2 * p + half - 4 * qb) * KB)
                                      if (causal and "nodiag" not in parts)
                                      else 0)
                                if es[h] is None:
                                    rhs = qt_sb[b][:, q0:q0 + QB]
                                    co = 0
                                else:
                                    rhs = es[h][:, half * QB + co:
                                                (half + 1) * QB]
                                nc.tensor.matmul(
                                    ps_o[h][:, co:QB],
                                    v_sb[h][:, b * NKB + kb, :],
                                    rhs,
                                    start=(kb == 0), stop=(kb == nkb - 1),
                                )

                    npair = nkb // 2
                    es_prev = scores_pair(0)
                    for p in range(1, npair):
                        es = scores_pair(p)
                        pv_pair(p - 1, es_prev)
                        es_prev = es
                    pv_pair(npair - 1, es_prev)

                    at = work.tile([128, QB], BF16, tag="attnT", name="attnT")
                    if "nonorm" in parts:
                        nc.vector.tensor_copy(at[0:DH, :], ps_o[0][0:DH, :])
                        nc.vector.tensor_copy(at[DH:2 * DH, :],
                                              ps_o[1][0:DH, :])
                    else:
                        # evacuate the PV accumulators (values + denominator
                        # row) to SBUF immediately so the psum banks free up
                        # for the next strip's PV; then the broadcast and the
                        # normalize multiplies run on the otherwise-idle
                        # GPSIMD engine so the DVE FIFO stays clear for the
                        # next strip's triangle-mask muls (which gate PV).
                        last = (b == B - 1 and qb == NQB - 1)
                        rcr = work.tile([1, 2 * QB], F32, tag="rcr",
                                        name="rcr")
                        if last:
                            # nothing competes for these psum banks after the
                            # final strip: skip the evacuation copies and
                            # normalize straight out of psum (shorter chain
                            # before the exchange + output projection)
                            for h in range(HPC):
                                nc.vector.reciprocal(
                                    rcr[0:1, h * QB:(h + 1) * QB],
                                    ps_o[h][DH:DH + 1, :])
                            vals = [ps_o[h][0:DH, :] for h in range(HPC)]
                        else:
                            pvs = work.tile([128, QB], F32, tag="pvs",
                                            name="pvs")
                            rc = work.tile([1, 2 * QB], F32, tag="rc",
                                           name="rc")
                            for h in range(HPC):
                                nc.vector.tensor_copy(
                                    pvs[h * DH:(h + 1) * DH, :],
                                    ps_o[h][0:DH, :])
                                nc.vector.tensor_copy(
                                    rc[0:1, h * QB:(h + 1) * QB],
                                    ps_o[h][DH:DH + 1, :])
                            nc.vector.reciprocal(rcr[:], rc[:])
                            vals = [pvs[h * DH:(h + 1) * DH, :]
                                    for h in range(HPC)]
                        rpb = work.tile([128, 2 * QB], F32, tag="rpb",
                                        name="rpb")
                        nc.gpsimd.partition_broadcast(rpb[:], rcr[0:1, :])
                        for h in range(HPC):
                            nc.vector.tensor_mul(
                                at[h * DH:(h + 1) * DH, :],
                                vals[h],
                                rpb[h * DH:(h + 1) * DH,
                                    h * QB:(h + 1) * QB])
                    j = b * NQB + qb
                    nc.sync.dma_start(a2a_in[j], at[:])
                    if a2a_local:
                        # timing stand-in for the exchange (collectives can't
                        # sit in control flow): the real kernel's data path is
                        # at -> a2a_in -> [AllToAll, charged separately by the
                        # harness] -> ao_sb, so the local build mirrors just
                        # the two endpoint DMAs
                        nc.sync.dma_start(ao_sb[:, j, :], a2a_in[j])
                    if b == 0:
                        # fill the strip-tail normalize window with deferred
                        # batch-1 projection work (exactly one unit: a second
                        # one overruns the idle window and stalls the next
                        # strip's exp stream)
                        feed()

            # ================= AllToAll + output projection ================
            if parts == "qkv+att":
                return
            if not a2a_local:
                nc.gpsimd.collective_compute(
                    "AllToAll", ALU.bypass,
                    replica_groups=[list(range(NCORES))],
                    ins=[a2a_in[:]], outs=[a2a_out[:]],
                )
                for j in range(NCORES):
                    nc.sync.dma_start(ao_sb[:, j, :], a2a_out[j])
            # 4 waves of 2 output chains each through the 3 rotating big
            # psum slots. The first 3 waves run their j=0..6 accumulation
            # matmuls up front (those shards arrived before the last strip)
            # and defer the j=7 matmuls to the end, so the PE keeps working
            # under the last strip's normalize/exchange tail instead of
            # head-of-line blocking on ao_sb[7].
            def opj_mm(ps, w, j, start, stop):
                for half in range(2):
                    ob = 2 * w + half
                    nc.tensor.matmul(
                        ps[:, half * RPC:(half + 1) * RPC],
                        wot_sb[:, j, ob * 128:(ob + 1) * 128],
                        ao_sb[:, j, :],
                        start=start, stop=stop,
                    )

            def opj_drain(ps, w):
                os = work.tile([128, 2 * RPC], F32, tag="os", name="os")
                for half in range(2):
                    ob = 2 * w + half
                    nc.vector.tensor_scalar(
                        os[:, half * RPC:(half + 1) * RPC],
                        ps[:, half * RPC:(half + 1) * RPC],
                        bo_sb[:, ob:ob + 1], None, ALU.add)
                    nc.sync.dma_start(out_t[ob * 128:(ob + 1) * 128, :],
                                      os[:, half * RPC:(half + 1) * RPC])

            waves = []
            for w in range(3):
                ps = big_ps(w, f"ps_outp{w}")
                for j in range(NCORES - 1):
                    opj_mm(ps, w, j, start=(j == 0), stop=False)
                waves.append(ps)
            for w in range(3):
                opj_mm(waves[w], w, NCORES - 1, start=False, stop=True)
                opj_drain(waves[w], w)
            ps = big_ps(3, "ps_outp3")
            for j in range(NCORES):
                opj_mm(ps, 3, j, start=(j == 0), stop=(j == NCORES - 1))
            opj_drain(ps, 3)

        if loop_n:
            with tc.For_i(0, loop_n, 1,
                          hint_engines=(mybir.EngineType.PE,
                                        mybir.EngineType.DVE,
                                        mybir.EngineType.Activation)):
                emit_body(a2a_local=True)
            dsb = persist.tile([1, 16], F32)
            nc.vector.memset(dsb[:], 0.0)
            nc.sync.dma_start(dummy[:], dsb[:])
        else:
            for _ in range(repeat):
                emit_body(a2a_local=a2a_local)

    nc.compile()
    return nc


def _build_a2a_bench(k: int):
    """k back-to-back AllToAlls on the kernel's exchange buffer size."""
    nc = bacc.Bacc("TRN2", target_bir_lowering=False, debug=False,
                   num_devices=NCORES)
    src = nc.dram_tensor("src", [NCORES, HD, RPC], BF16,
                         kind="ExternalInput").ap()
    dst = nc.dram_tensor("dst", [1, 16], F32, kind="ExternalOutput").ap()
    with tile.TileContext(nc) as tc, ExitStack() as octx:
        dram = octx.enter_context(tc.tile_pool(name="dram", bufs=1,
                                               space="DRAM"))
        pool = octx.enter_context(tc.tile_pool(name="sb", bufs=1))
        a = dram.tile([NCORES, HD, RPC], BF16)
        bb = dram.tile([NCORES, HD, RPC], BF16)
        nc.sync.dma_start(a[:], src[:])
        bufs = [a, bb]
        for i in range(k):
            nc.gpsimd.collective_compute(
                "AllToAll", ALU.bypass,
                replica_groups=[list(range(NCORES))],
                ins=[bufs[i % 2][:]], outs=[bufs[(i + 1) % 2][:]],
            )
        dsb = pool.tile([1, 16], F32)
        nc.vector.memset(dsb[:], 0.0)
        nc.sync.dma_start(dst[:], dsb[:])
    nc.compile()
    return nc


def _make_runner(nc):
    """Jitted 8-core SPMD executor for a compiled Bass module."""
    install_neuronx_cc_hook()
    partition_name = nc.partition_id_tensor.name if nc.partition_id_tensor else None
    in_names, out_names, out_avals = [], [], []
    for alloc in nc.m.functions[0].allocations:
        if not isinstance(alloc, mybir.MemoryLocationSet):
            continue
        name = alloc.memorylocations[0].name
        if alloc.kind == "ExternalInput":
            if name != partition_name:
                in_names.append(name)
        elif alloc.kind == "ExternalOutput":
            out_names.append(name)
            out_avals.append(jax.core.ShapedArray(
                tuple(alloc.tensor_shape), mybir.dt.np(alloc.dtype)))
    n_params = len(in_names)
    n_outs = len(out_avals)
    all_in_names = list(in_names) + list(out_names)
    if partition_name is not None:
        all_in_names.append(partition_name)
    donate = tuple(range(n_params, n_params + n_outs))

    def _body(*args):
        operands = list(args)
        if partition_name is not None:
            operands.append(partition_id_tensor())
        return tuple(_bass_exec_p.bind(
            *operands,
            out_avals=tuple(out_avals),
            in_names=tuple(all_in_names),
            out_names=tuple(out_names),
            lowering_input_output_aliases=(),
            sim_require_finite=True,
            sim_require_nnan=True,
            nc=nc,
        ))

    devices = jax.devices()[:NCORES]
    mesh = Mesh(np.asarray(devices), ("core",))
    sharded = jax.jit(
        shard_map(_body, mesh=mesh,
                  in_specs=(PartitionSpec("core"),) * (n_params + n_outs),
                  out_specs=(PartitionSpec("core"),) * n_outs,
                  check_rep=False),
        donate_argnums=donate, keep_unused=True)

    zero_shapes = [a.shape for a in out_avals]
    zero_dtypes = [a.dtype for a in out_avals]

    def _zeros():
        return [np.zeros((NCORES * s[0], *s[1:]), d)
                for s, d in zip(zero_shapes, zero_dtypes)]

    def prepare(in_maps):
        """Concatenate per-core inputs and stage them on device once."""
        return [
            jax.device_put(np.concatenate(
                [np.asarray(m[name]) for m in in_maps], axis=0))
            for name in in_names
        ]

    def run_prepared(handles, as_numpy=True):
        out_arrs = sharded(*handles, *_zeros())
        if not as_numpy:
            jax.block_until_ready(out_arrs)
            return out_arrs
        return [
            {name: np.asarray(out_arrs[i]).reshape(NCORES, *zero_shapes[i])[c]
             for i, name in enumerate(out_names)}
            for c in range(NCORES)
        ]

    def run(in_maps):
        return run_prepared(prepare(in_maps))

    run.prepare = prepare
    run.run_prepared = run_prepared
    return run


def _shard_inputs(x, Wi, bi, Wo, bo, causal):
    """Host-side slicing/layout prep -> per-core input maps."""
    bf = ml_dtypes.bfloat16
    x = np.asarray(x, np.float32)
    Wi = np.asarray(Wi, np.float32)
    bi = np.asarray(bi, np.float32)
    Wo = np.asarray(Wo, np.float32)
    bo = np.asarray(bo, np.float32)

    xt = np.ascontiguousarray(x.reshape(R, D).T).astype(bf)       # (D, R)
    wot = np.ascontiguousarray(Wo.T).astype(bf)                   # (D, D)

    if causal:
        # additive mask pattern, accumulated onto diagonal score sub-blocks
        # via matmul(identity, m): scores[k, q] += m[k, q]; masked (k > q)
        # positions get -1e9 so exp underflows to exactly 0
        i = np.arange(KB)[:, None]
        j = np.arange(KB)[None, :]
        m = np.where(j >= i, 0.0, -1e9).astype(bf)
    else:
        m = np.zeros((KB, KB), bf)

    in_maps = []
    for c in range(NCORES):
        rows = np.concatenate([
            np.arange(c * HD, (c + 1) * HD),
            D + np.arange(c * HD, (c + 1) * HD),
            2 * D + np.arange(c * HD, (c + 1) * HD),
        ])
        wit_c = np.ascontiguousarray(Wi[rows].T).astype(bf)       # (D, 384)
        bi_c = np.ascontiguousarray(bi[rows]).astype(np.float32)  # (384,)
        in_maps.append({
            "xt": xt, "wit": wit_c, "bi_s": bi_c,
            "wot": wot, "bo_f": bo, "masks": m,
        })
    return in_maps


_CACHE = {}


def _get_runner(causal, repeat=1):
    key = (causal, repeat)
    if key not in _CACHE:
        nc = _build(causal, repeat)
        _CACHE[key] = _make_runner(nc)
    return _CACHE[key]


def kernel(x, Wi, bi, Wo, bo, causal_mask):
    causal = bool(int(np.asarray(causal_mask)))
    run = _get_runner(causal)
    in_maps = _shard_inputs(x, Wi, bi, Wo, bo, causal)
    res = run(in_maps)
    # res[c]["out_t"]: (D, RPC) fp32 = transposed rows [c*RPC, (c+1)*RPC)
    full = np.concatenate([res[c]["out_t"].T for c in range(NCORES)], axis=0)
    return np.ascontiguousarray(full.reshape(B, S, D).astype(np.float32))

